# revision 13
# baseline (speedup 1.0000x reference)
import numpy as np
import ml_dtypes

B, H, N, D, M = 4, 12, 8192, 64, 128
NCORES = 8
PAIRS = (B * H) // NCORES   # 6 pairs per core
NG2 = 8                     # 8 double-groups of 1024 per pair

_cache = {}


def _build():
    if "nc" in _cache:
        return _cache["nc"]
    import concourse.bacc as bacc
    import concourse.mybir as mybir
    import concourse.tile as tile

    f32 = mybir.dt.float32
    f16 = mybir.dt.float16
    bf16 = mybir.dt.bfloat16
    AF = mybir.ActivationFunctionType

    nc = bacc.Bacc("TRN2", target_bir_lowering=False, debug=False)
    # K^T on partitions 0:64, Q^T on 64:128
    KQT = nc.declare_dram_parameter("KQT", [PAIRS, 128, N], f16, isOutput=False)
    V65 = nc.declare_dram_parameter("V65", [PAIRS, N, 65], bf16, isOutput=False)
    # landmark tiles: cols 0:128 = nr^T, 128:256 = nc^T
    # rows 64:128 duplicate rows 0:64 so matmuls can match either operand base
    LANDH = nc.declare_dram_parameter("LANDH", [PAIRS, 128, 256], f16, isOutput=False)
    LAND32 = nc.declare_dram_parameter("LAND32", [PAIRS, 64, 256], f32, isOutput=False)
    GS = nc.declare_dram_parameter("GS", [1, 1], f32, isOutput=False)
    # X^T output: rows 0:64 numerator, row 64 denominator
    XOT = nc.declare_dram_parameter("XOT", [PAIRS, 65, N], f32, isOutput=True)

    with tile.TileContext(nc) as tc:
        with (tc.tile_pool(name="pc", bufs=1) as pc,
              tc.tile_pool(name="pkq", bufs=4) as pkq,
              tc.tile_pool(name="pv", bufs=2) as pv,
              tc.tile_pool(name="pw", bufs=2) as pw,
              tc.tile_pool(name="pxs", bufs=2) as pxs,
              tc.tile_pool(name="pns", bufs=2) as pns,
              tc.tile_pool(name="ps_rt", bufs=1, space="PSUM") as ps_rt_pool,
              tc.tile_pool(name="ps_S", bufs=1, space="PSUM") as ps_S_pool,
              tc.tile_pool(name="ps_cm", bufs=1, space="PSUM") as ps_cm_pool,
              tc.tile_pool(name="ps_x", bufs=1, space="PSUM") as ps_x_pool,
              tc.tile_pool(name="ps_ns", bufs=1, space="PSUM") as ps_ns_pool):

            # ---- bulk loads for pair 0 first (critical path) ----
            kq_tiles = [None] * PAIRS
            v_tiles = [None] * PAIRS

            def load_pair(p):
                kq = pkq.tile([128, N], f16, tag="kq", name="kq")
                nc.gpsimd.dma_start(kq[:], KQT[p])
                vt = pv.tile([128, N // 128, 65], bf16, tag="v65", name="vt")
                nc.gpsimd.dma_start(vt[:], V65[p].rearrange("(t pp) d -> pp t d", pp=128))
                kq_tiles[p] = kq
                v_tiles[p] = vt

            load_pair(0)

            # ---- constants ----
            ident = pc.tile([128, 128], bf16, tag="ident")
            nc.gpsimd.memset(ident[:], 0.0)
            nc.gpsimd.affine_select(out=ident[:], in_=ident[:],
                compare_op=mybir.AluOpType.not_equal, fill=1.0, base=0,
                pattern=[[-1, 128]], channel_multiplier=1)
            diags = {}
            for val, tg in ((7.0, "i7"), (15.0, "i15"), (13.0, "i13")):
                t = pc.tile([128, PAIRS * 128], bf16, tag=tg, name=tg)
                nc.gpsimd.memset(t[:], 0.0)
                for p in range(PAIRS):
                    nc.gpsimd.affine_select(
                        out=t[:, p * 128:(p + 1) * 128],
                        in_=t[:, p * 128:(p + 1) * 128],
                        compare_op=mybir.AluOpType.not_equal, fill=val, base=0,
                        pattern=[[-1, 128]], channel_multiplier=1)
                diags[tg] = t
            i7b, i15b, i13b = diags["i7"], diags["i15"], diags["i13"]

            ones_row = pc.tile([1, 128], f32, tag="ones_row")
            nc.vector.memset(ones_row[:], 1.0)
            gs_sb = pc.tile([1, 1], f32, tag="gs_sb")
            nc.sync.dma_start(gs_sb[:], GS[:])
            ps_bc = ps_cm_pool.tile([128, 1], f32, tag="ps_cm", name="ps_bc")
            nc.tensor.matmul(ps_bc[:], ones_row[:], gs_sb[:], start=True, stop=True)
            gsb = pc.tile([128, 1], f32, tag="gsb")
            nc.scalar.copy(gsb[:], ps_bc[:])

            # ---- landmark loads (small, sync queue) ----
            landh = []
            land32 = []
            for p in range(PAIRS):
                lh = pc.tile([128, 256], f16, tag=f"landh{p}", name="lh")
                l32 = pc.tile([64, 256], f32, tag=f"land32{p}", name="l32")
                nc.sync.dma_start(lh[:], LANDH[p])
                nc.sync.dma_start(l32[:], LAND32[p])
                landh.append(lh)
                land32.append(l32)

            load_pair(1)

            # ---- m-chains: k2 for all pairs into batched tiles ----
            W = PAIRS * 128
            k2_all = pc.tile([128, W], bf16, tag="k2_all")
            for p in range(PAIRS):
                ps_m = ps_cm_pool.tile([128, 128], f32, tag="ps_cm", name="ps_m")
                nc.tensor.matmul(ps_m[:], land32[p][:, 0:128], land32[p][:, 128:256],
                                 start=True, stop=True)
                e_m = pns.tile([128, 128], f32, tag="e_m", name="e_m")
                msum = pns.tile([128, 1], f32, tag="msum", name="msum")
                nc.scalar.activation(e_m[:], ps_m[:], AF.Exp, accum_out=msum[:])
                mrec = pns.tile([128, 1], f32, tag="mrec", name="mrec")
                nc.vector.reciprocal(mrec[:], msum[:])
                nc.vector.tensor_scalar_mul(k2_all[:, p * 128:(p + 1) * 128],
                                            e_m[:], mrec[:])
            # k2t + NS init: Vm0 = gs*k2^T, Vt0 = gs*k2
            ps_t = ps_ns_pool.tile([128, W], bf16, tag="ps_ns", name="ps_t")
            for p in range(PAIRS):
                nc.tensor.transpose(ps_t[:, p * 128:(p + 1) * 128],
                                    k2_all[:, p * 128:(p + 1) * 128], ident[:])
            k2t_all = pc.tile([128, W], bf16, tag="k2t_all")
            nc.scalar.copy(k2t_all[:], ps_t[:])
            vm_all = pns.tile([128, W], bf16, tag="vm", name="vm0")
            nc.vector.tensor_scalar_mul(vm_all[:], ps_t[:], gsb[:])
            vt_all = pns.tile([128, W], bf16, tag="vt", name="vt0")
            nc.vector.tensor_scalar_mul(vt_all[:], k2_all[:], gsb[:])

            ns_state = {"vm": vm_all, "vt": vt_all}

            # ---- batched NS stage thunks (drip-fed into slots) ----
            def ns_thunks():
                for _ in range(6):
                    carry = {}

                    def cA():
                        ps_P = ps_ns_pool.tile([128, W], f32, tag="ps_ns", name="ps_P")
                        vm = ns_state["vm"]
                        for p in range(PAIRS):
                            nc.tensor.matmul(ps_P[:, p * 128:(p + 1) * 128],
                                             k2t_all[:, p * 128:(p + 1) * 128],
                                             vm[:, p * 128:(p + 1) * 128],
                                             start=True, stop=True)
                        pbf = pns.tile([128, W], bf16, tag="pbf", name="pbf")
                        nc.scalar.copy(pbf[:], ps_P[:])
                        t1 = pns.tile([128, W], bf16, tag="t1", name="t1")
                        nc.vector.tensor_sub(t1[:], i7b[:], ps_P[:])
                        carry["pbf"] = pbf
                        carry["t1"] = t1

                    def cB():
                        ps_pt = ps_ns_pool.tile([128, W], bf16, tag="ps_ns", name="ps_pt")
                        for p in range(PAIRS):
                            nc.tensor.transpose(ps_pt[:, p * 128:(p + 1) * 128],
                                                carry["pbf"][:, p * 128:(p + 1) * 128],
                                                ident[:])
                        ptb = pns.tile([128, W], bf16, tag="ptb", name="ptb")
                        nc.scalar.copy(ptb[:], ps_pt[:])
                        carry["ptb"] = ptb

                    def cC():
                        ps_u = ps_ns_pool.tile([128, W], f32, tag="ps_ns", name="ps_u")
                        for p in range(PAIRS):
                            nc.tensor.matmul(ps_u[:, p * 128:(p + 1) * 128],
                                             carry["ptb"][:, p * 128:(p + 1) * 128],
                                             carry["t1"][:, p * 128:(p + 1) * 128],
                                             start=True, stop=True)
                        t2 = pns.tile([128, W], bf16, tag="t2", name="t2")
                        nc.vector.tensor_sub(t2[:], i15b[:], ps_u[:])
                        carry["t2"] = t2

                    def cD():
                        ps_w = ps_ns_pool.tile([128, W], f32, tag="ps_ns", name="ps_w")
                        for p in range(PAIRS):
                            nc.tensor.matmul(ps_w[:, p * 128:(p + 1) * 128],
                                             carry["ptb"][:, p * 128:(p + 1) * 128],
                                             carry["t2"][:, p * 128:(p + 1) * 128],
                                             start=True, stop=True)
                        t3 = pns.tile([128, W], bf16, tag="t3", name="t3")
                        nc.vector.tensor_sub(t3[:], i13b[:], ps_w[:])
                        carry["t3"] = t3

                    def cE():
                        vt = ns_state["vt"]
                        ps_v = ps_ns_pool.tile([128, W], f32, tag="ps_ns", name="ps_v")
                        for p in range(PAIRS):
                            nc.tensor.matmul(ps_v[:, p * 128:(p + 1) * 128],
                                             vt[:, p * 128:(p + 1) * 128],
                                             carry["t3"][:, p * 128:(p + 1) * 128],
                                             start=True, stop=True)
                        vm_n = pns.tile([128, W], bf16, tag="vm", name="vm_n")
                        nc.vector.tensor_scalar(vm_n[:], ps_v[:], 0.25, scalar2=None,
                                                op0=mybir.AluOpType.mult)
                        ps_vt = ps_ns_pool.tile([128, W], f32, tag="ps_ns", name="ps_vt")
                        for p in range(PAIRS):
                            nc.tensor.matmul(ps_vt[:, p * 128:(p + 1) * 128],
                                             carry["t3"][:, p * 128:(p + 1) * 128],
                                             vt[:, p * 128:(p + 1) * 128],
                                             start=True, stop=True)
                        vt_n = pns.tile([128, W], bf16, tag="vt", name="vt_n")
                        nc.vector.tensor_scalar(vt_n[:], ps_vt[:], 0.25, scalar2=None,
                                                op0=mybir.AluOpType.mult)
                        ns_state["vm"] = vm_n
                        ns_state["vt"] = vt_n

                    yield cA
                    yield cB
                    yield cC
                    yield cD
                    yield cE

            ns_iter = ns_thunks()

            # ---- main pipelined slots: p1(s) for s<6, p3(s-2) for s>=2 ----
            A1 = [None] * PAIRS
            ps_S_handle = [None] * PAIRS
            s_bf_handle = [None] * PAIRS
            ert_prev = [None]
            ec_prev = [None]
            xstage = [None]

            for s in range(PAIRS + 2):
                if 2 <= s + 1 <= PAIRS - 1:
                    load_pair(s + 1)
                for gg in range(NG2 + 1):
                    # p1: r^T matmuls (pair s, group gg)
                    if s < PAIRS and gg < NG2:
                        kq = kq_tiles[s]
                        ps_rt = ps_rt_pool.tile([128, 1024], f32, tag="ps_rt",
                                                name="ps_rt")
                        for j in range(8):
                            nc.tensor.matmul(
                                ps_rt[:, j * 128:(j + 1) * 128],
                                kq[0:64, gg * 1024 + j * 128: gg * 1024 + (j + 1) * 128],
                                landh[s][0:64, 0:128],
                                start=True, stop=True)
                        ert = pw.tile([128, 1024], bf16, tag="ert", name="ert")
                        nc.scalar.activation(ert[:], ps_rt[:], AF.Exp)
                        ert_new = ert
                    # p3: first X^T matmul (pair s-2, group gg-1)
                    if s >= 2 and gg >= 1:
                        pp = s - 2
                        gm = gg - 1
                        if gm % 2 == 0:
                            xstage[0] = pxs.tile([65, 2048], f32, tag="xstage",
                                                 name="xstage")
                        ps_x0 = ps_x_pool.tile([65, 512], f32, tag="ps_x", name="ps_x0")
                        nc.tensor.matmul(ps_x0[:], A1[pp][:], ec_prev[0][:, 0:512],
                                         start=True, stop=True)
                        nc.vector.tensor_copy(
                            xstage[0][:, (gm % 2) * 1024:(gm % 2) * 1024 + 512],
                            ps_x0[:])
                    # p3: c matmuls + exp (pair s-2, group gg)
                    if s >= 2 and gg < NG2:
                        pp = s - 2
                        kqp = kq_tiles[pp]
                        ps_c = ps_cm_pool.tile([128, 1024], f32, tag="ps_cm",
                                               name="ps_c")
                        for h2 in range(2):
                            nc.tensor.matmul(
                                ps_c[:, h2 * 512:(h2 + 1) * 512],
                                landh[pp][64:128, 128:256],
                                kqp[64:128, gg * 1024 + h2 * 512: gg * 1024 + (h2 + 1) * 512],
                                start=True, stop=True)
                        ec = pw.tile([128, 1024], bf16, tag="ec", name="ec")
                        nc.scalar.activation(ec[:], ps_c[:], AF.Exp)
                        ec_new = ec
                    # p1: S accumulation (pair s, group gg-1)
                    if s < PAIRS and gg >= 1:
                        if gg == 1:
                            ps_S = ps_S_pool.tile([128, 65], f32, tag="ps_S",
                                                  name="ps_S")
                            ps_S_handle[s] = ps_S
                        ps_S = ps_S_handle[s]
                        for j in range(8):
                            nc.tensor.matmul(
                                ps_S[:],
                                ert_prev[0][:, j * 128:(j + 1) * 128],
                                v_tiles[s][:, (gg - 1) * 8 + j, :],
                                start=(gg == 1 and j == 0),
                                stop=(gg == NG2 and j == 7),
                                skip_group_check=True)
                    # p3: second X^T matmul + copy + store (pair s-2, group gg-1)
                    if s >= 2 and gg >= 1:
                        pp = s - 2
                        gm = gg - 1
                        ps_x1 = ps_x_pool.tile([65, 512], f32, tag="ps_x", name="ps_x1")
                        nc.tensor.matmul(ps_x1[:], A1[pp][:], ec_prev[0][:, 512:1024],
                                         start=True, stop=True)
                        nc.vector.tensor_copy(
                            xstage[0][:, (gm % 2) * 1024 + 512:(gm % 2) * 1024 + 1024],
                            ps_x1[:])
                        if gm % 2 == 1:
                            q = gm // 2
                            nc.sync.dma_start(
                                XOT[pp, :, q * 2048:(q + 1) * 2048], xstage[0][:])
                    if s < PAIRS and gg < NG2:
                        ert_prev[0] = ert_new
                    if s >= 2 and gg < NG2:
                        ec_prev[0] = ec_new
                    # drip-feed two NS stages per gg (emission done by slot 1)
                    for _ in range(2):
                        th = next(ns_iter, None)
                        if th is not None:
                            th()

                # S normalization for pair s (frees ps_S promptly; no NS dep)
                if s < PAIRS:
                    ps_S = ps_S_handle[s]
                    rrec = pw.tile([128, 1], f32, tag="rrec", name="rrec")
                    nc.vector.reciprocal(rrec[:], ps_S[:, 64:65])
                    s_bf = pw.tile([128, 64], bf16, tag="s_bf", name="s_bf")
                    nc.vector.tensor_scalar_mul(s_bf[:], ps_S[:, 0:64], rrec[:])
                    s_bf_handle[s] = s_bf
                # A-matmul for pair s-1 (needs final NS state, emitted by slot 1)
                if 1 <= s <= PAIRS:
                    pa = s - 1
                    vt_fin = ns_state["vt"]
                    ps_A = ps_x_pool.tile([128, 64], f32, tag="ps_x", name="ps_A")
                    nc.tensor.matmul(ps_A[:], vt_fin[:, pa * 128:(pa + 1) * 128],
                                     s_bf_handle[pa][:], start=True, stop=True)
                    a1 = pw.tile([128, 65], bf16, tag="A1", bufs=3, name="a1")
                    nc.vector.memset(a1[:, 64:65], 1.0)
                    nc.vector.tensor_copy(a1[:, 0:64], ps_A[:])
                    A1[pa] = a1

    nc.finalize()
    _cache["nc"] = nc
    return nc


def kernel(Q, K, V, mask):
    from concourse.bass_utils import run_bass_kernel_spmd

    Q = np.asarray(Q, dtype=np.float32)
    K = np.asarray(K, dtype=np.float32)
    V = np.asarray(V, dtype=np.float32)
    BH = B * H
    Qf = Q.reshape(BH, N, D)
    Kf = K.reshape(BH, N, D)
    Vf = V.reshape(BH, N, D)

    # host: top-k selection + global NS init scalar
    landh = np.empty((BH, 128, 256), np.float16)
    land32 = np.empty((BH, 64, 256), np.float32)
    gmax = 0.0
    for i in range(BH):
        sK = Kf[i, :, 0].copy(); sK[0] = np.inf
        iK = np.sort(np.argpartition(-sK, M)[:M])
        sQ = Qf[i, :, 0].copy(); sQ[0] = np.inf
        iQ = np.sort(np.argpartition(-sQ, M)[:M])
        nr = Qf[i, iQ]          # [M, D]
        ncm = Kf[i, iK]         # [M, D]
        land32[i, :, 0:128] = nr.T
        land32[i, :, 128:256] = ncm.T
        landh[i, 0:64, 0:128] = nr.T
        landh[i, 0:64, 128:256] = ncm.T
        landh[i, 64:128, :] = landh[i, 0:64, :]
        md = nr.astype(np.float64) @ ncm.astype(np.float64).T
        e = np.exp(md - md.max(axis=1, keepdims=True))
        k2 = e / e.sum(axis=1, keepdims=True)
        gmax = max(gmax, float(k2.sum(axis=0).max()))

    kqt = np.empty((BH, 128, N), np.float16)
    kqt[:, 0:64, :] = Kf.transpose(0, 2, 1)
    kqt[:, 64:128, :] = Qf.transpose(0, 2, 1)
    v65 = np.empty((BH, N, 65), ml_dtypes.bfloat16)
    v65[:, :, 0:64] = Vf.astype(ml_dtypes.bfloat16)
    v65[:, :, 64] = 1.0
    gs = np.array([[1.0 / gmax]], np.float32)

    nc = _build()
    in_maps = []
    for c in range(NCORES):
        sl = slice(c * PAIRS, (c + 1) * PAIRS)
        in_maps.append({"KQT": kqt[sl], "V65": v65[sl],
                        "LANDH": landh[sl], "LAND32": land32[sl], "GS": gs})
    res = run_bass_kernel_spmd(nc, in_maps, list(range(NCORES)))
    global LAST_RESULTS
    LAST_RESULTS = res
    xot = np.concatenate([res.results[c]["XOT"] for c in range(NCORES)], axis=0)
    X = xot[:, 0:64, :] / xot[:, 64:65, :]
    return np.ascontiguousarray(X.transpose(0, 2, 1)).reshape(B, H, N, D).astype(np.float32)


# revision 16
# speedup vs baseline: 1.1725x; 1.1725x over previous
import numpy as np
import ml_dtypes

B, H, N, D, M = 4, 12, 8192, 64, 128
NCORES = 8
PAIRS = (B * H) // NCORES   # 6 pairs per core
NG2 = 8                     # 8 double-groups of 1024 per pair

_cache = {}


def _build():
    if "nc" in _cache:
        return _cache["nc"]
    import concourse.bacc as bacc
    import concourse.mybir as mybir
    import concourse.tile as tile

    f32 = mybir.dt.float32
    f16 = mybir.dt.float16
    bf16 = mybir.dt.bfloat16
    AF = mybir.ActivationFunctionType

    nc = bacc.Bacc("TRN2", target_bir_lowering=False, debug=False)
    # K^T on partitions 0:64, Q^T on 64:128
    KQT = nc.declare_dram_parameter("KQT", [PAIRS, 128, N], f16, isOutput=False)
    V65 = nc.declare_dram_parameter("V65", [PAIRS, N, 65], bf16, isOutput=False)
    # landmark tiles: cols 0:128 = nr^T, 128:256 = nc^T
    # rows 64:128 duplicate rows 0:64 so matmuls can match either operand base
    LANDH = nc.declare_dram_parameter("LANDH", [PAIRS, 128, 256], f16, isOutput=False)
    LAND32 = nc.declare_dram_parameter("LAND32", [PAIRS, 64, 256], f32, isOutput=False)
    GS = nc.declare_dram_parameter("GS", [1, 1], f32, isOutput=False)
    # X^T output: rows 0:64 numerator, row 64 denominator
    XOT = nc.declare_dram_parameter("XOT", [PAIRS, 65, N], f32, isOutput=True)

    with tile.TileContext(nc) as tc:
        with (tc.tile_pool(name="pc", bufs=1) as pc,
              tc.tile_pool(name="pkq", bufs=4) as pkq,
              tc.tile_pool(name="pv", bufs=2) as pv,
              tc.tile_pool(name="pw", bufs=2) as pw,
              tc.tile_pool(name="pxs", bufs=2) as pxs,
              tc.tile_pool(name="pns", bufs=2) as pns,
              tc.tile_pool(name="ps_rt", bufs=1, space="PSUM") as ps_rt_pool,
              tc.tile_pool(name="ps_S", bufs=1, space="PSUM") as ps_S_pool,
              tc.tile_pool(name="ps_cm", bufs=1, space="PSUM") as ps_cm_pool,
              tc.tile_pool(name="ps_x", bufs=1, space="PSUM") as ps_x_pool,
              tc.tile_pool(name="ps_ns", bufs=1, space="PSUM") as ps_ns_pool):

            # ---- bulk loads for pair 0 first (critical path) ----
            kq_tiles = [None] * PAIRS
            v_tiles = [None] * PAIRS

            def load_pair(p):
                kq = pkq.tile([128, N], f16, tag="kq", name="kq")
                nc.gpsimd.dma_start(kq[:], KQT[p])
                vt = pv.tile([128, N // 128, 65], bf16, tag="v65", name="vt")
                nc.gpsimd.dma_start(vt[:], V65[p].rearrange("(t pp) d -> pp t d", pp=128))
                kq_tiles[p] = kq
                v_tiles[p] = vt

            load_pair(0)

            # ---- constants ----
            ident = pc.tile([128, 128], bf16, tag="ident")
            nc.gpsimd.memset(ident[:], 0.0)
            nc.gpsimd.affine_select(out=ident[:], in_=ident[:],
                compare_op=mybir.AluOpType.not_equal, fill=1.0, base=0,
                pattern=[[-1, 128]], channel_multiplier=1)
            diags = {}
            for val, tg in ((7.0, "i7"), (15.0, "i15"), (13.0, "i13")):
                t = pc.tile([128, PAIRS * 128], bf16, tag=tg, name=tg)
                nc.gpsimd.memset(t[:], 0.0)
                for p in range(PAIRS):
                    nc.gpsimd.affine_select(
                        out=t[:, p * 128:(p + 1) * 128],
                        in_=t[:, p * 128:(p + 1) * 128],
                        compare_op=mybir.AluOpType.not_equal, fill=val, base=0,
                        pattern=[[-1, 128]], channel_multiplier=1)
                diags[tg] = t
            i7b, i15b, i13b = diags["i7"], diags["i15"], diags["i13"]

            ones_row = pc.tile([1, 128], f32, tag="ones_row")
            nc.vector.memset(ones_row[:], 1.0)
            gs_sb = pc.tile([1, 1], f32, tag="gs_sb")
            nc.sync.dma_start(gs_sb[:], GS[:])
            ps_bc = ps_cm_pool.tile([128, 1], f32, tag="ps_cm", name="ps_bc")
            nc.tensor.matmul(ps_bc[:], ones_row[:], gs_sb[:], start=True, stop=True)
            gsb = pc.tile([128, 1], f32, tag="gsb")
            nc.scalar.copy(gsb[:], ps_bc[:])

            # ---- landmark loads (small, sync queue) ----
            landh = []
            land32 = []
            for p in range(PAIRS):
                lh = pc.tile([128, 256], f16, tag=f"landh{p}", name="lh")
                l32 = pc.tile([64, 256], f32, tag=f"land32{p}", name="l32")
                nc.sync.dma_start(lh[:], LANDH[p])
                nc.sync.dma_start(l32[:], LAND32[p])
                landh.append(lh)
                land32.append(l32)

            load_pair(1)

            # ---- m-chains: k2 for all pairs into batched tiles ----
            W = PAIRS * 128
            k2_all = pc.tile([128, W], bf16, tag="k2_all")
            for p in range(PAIRS):
                ps_m = ps_cm_pool.tile([128, 128], f32, tag="ps_cm", name="ps_m")
                nc.tensor.matmul(ps_m[:], land32[p][:, 0:128], land32[p][:, 128:256],
                                 start=True, stop=True)
                e_m = pns.tile([128, 128], f32, tag="e_m", name="e_m")
                msum = pns.tile([128, 1], f32, tag="msum", name="msum")
                nc.scalar.activation(e_m[:], ps_m[:], AF.Exp, accum_out=msum[:])
                mrec = pns.tile([128, 1], f32, tag="mrec", name="mrec")
                nc.vector.reciprocal(mrec[:], msum[:])
                nc.vector.tensor_scalar_mul(k2_all[:, p * 128:(p + 1) * 128],
                                            e_m[:], mrec[:])
            # k2t + NS init: Vm0 = gs*k2^T, Vt0 = gs*k2
            ps_t = ps_ns_pool.tile([128, W], bf16, tag="ps_ns", name="ps_t")
            for p in range(PAIRS):
                nc.tensor.transpose(ps_t[:, p * 128:(p + 1) * 128],
                                    k2_all[:, p * 128:(p + 1) * 128], ident[:])
            k2t_all = pc.tile([128, W], bf16, tag="k2t_all")
            nc.scalar.copy(k2t_all[:], ps_t[:])
            vm_all = pns.tile([128, W], bf16, tag="vm", name="vm0")
            nc.vector.tensor_scalar_mul(vm_all[:], ps_t[:], gsb[:])
            vt_all = pns.tile([128, W], bf16, tag="vt", name="vt0")
            nc.vector.tensor_scalar_mul(vt_all[:], k2_all[:], gsb[:])

            ns_state = {"vm": vm_all, "vt": vt_all}

            # ---- batched NS stage thunks (drip-fed into slots) ----
            def ns_thunks():
                for _ in range(6):
                    carry = {}

                    def cA():
                        ps_P = ps_ns_pool.tile([128, W], f32, tag="ps_ns", name="ps_P")
                        vm = ns_state["vm"]
                        for p in range(PAIRS):
                            nc.tensor.matmul(ps_P[:, p * 128:(p + 1) * 128],
                                             k2t_all[:, p * 128:(p + 1) * 128],
                                             vm[:, p * 128:(p + 1) * 128],
                                             start=True, stop=True)
                        pbf = pns.tile([128, W], bf16, tag="pbf", name="pbf")
                        nc.scalar.copy(pbf[:], ps_P[:])
                        t1 = pns.tile([128, W], bf16, tag="t1", name="t1")
                        nc.vector.tensor_sub(t1[:], i7b[:], ps_P[:])
                        carry["pbf"] = pbf
                        carry["t1"] = t1

                    def cB():
                        ps_pt = ps_ns_pool.tile([128, W], bf16, tag="ps_ns", name="ps_pt")
                        for p in range(PAIRS):
                            nc.tensor.transpose(ps_pt[:, p * 128:(p + 1) * 128],
                                                carry["pbf"][:, p * 128:(p + 1) * 128],
                                                ident[:])
                        ptb = pns.tile([128, W], bf16, tag="ptb", name="ptb")
                        nc.scalar.copy(ptb[:], ps_pt[:])
                        carry["ptb"] = ptb

                    def cC():
                        ps_u = ps_ns_pool.tile([128, W], f32, tag="ps_ns", name="ps_u")
                        for p in range(PAIRS):
                            nc.tensor.matmul(ps_u[:, p * 128:(p + 1) * 128],
                                             carry["ptb"][:, p * 128:(p + 1) * 128],
                                             carry["t1"][:, p * 128:(p + 1) * 128],
                                             start=True, stop=True)
                        t2 = pns.tile([128, W], bf16, tag="t2", name="t2")
                        nc.vector.tensor_sub(t2[:], i15b[:], ps_u[:])
                        carry["t2"] = t2

                    def cD():
                        ps_w = ps_ns_pool.tile([128, W], f32, tag="ps_ns", name="ps_w")
                        for p in range(PAIRS):
                            nc.tensor.matmul(ps_w[:, p * 128:(p + 1) * 128],
                                             carry["ptb"][:, p * 128:(p + 1) * 128],
                                             carry["t2"][:, p * 128:(p + 1) * 128],
                                             start=True, stop=True)
                        t3 = pns.tile([128, W], bf16, tag="t3", name="t3")
                        nc.vector.tensor_sub(t3[:], i13b[:], ps_w[:])
                        carry["t3"] = t3

                    def cE():
                        vt = ns_state["vt"]
                        ps_v = ps_ns_pool.tile([128, W], f32, tag="ps_ns", name="ps_v")
                        for p in range(PAIRS):
                            nc.tensor.matmul(ps_v[:, p * 128:(p + 1) * 128],
                                             vt[:, p * 128:(p + 1) * 128],
                                             carry["t3"][:, p * 128:(p + 1) * 128],
                                             start=True, stop=True)
                        vm_n = pns.tile([128, W], bf16, tag="vm", name="vm_n")
                        nc.vector.tensor_scalar(vm_n[:], ps_v[:], 0.25, scalar2=None,
                                                op0=mybir.AluOpType.mult)
                        ps_vt = ps_ns_pool.tile([128, W], f32, tag="ps_ns", name="ps_vt")
                        for p in range(PAIRS):
                            nc.tensor.matmul(ps_vt[:, p * 128:(p + 1) * 128],
                                             carry["t3"][:, p * 128:(p + 1) * 128],
                                             vt[:, p * 128:(p + 1) * 128],
                                             start=True, stop=True)
                        vt_n = pns.tile([128, W], bf16, tag="vt", name="vt_n")
                        nc.vector.tensor_scalar(vt_n[:], ps_vt[:], 0.25, scalar2=None,
                                                op0=mybir.AluOpType.mult)
                        ns_state["vm"] = vm_n
                        ns_state["vt"] = vt_n

                    yield cA
                    yield cB
                    yield cC
                    yield cD
                    yield cE

            ns_iter = ns_thunks()

            # ---- main pipelined slots: p1(s) for s<6, p3(s-2) for s>=2 ----
            A1 = [None] * PAIRS
            ps_S_handle = [None] * PAIRS
            s_bf_handle = [None] * PAIRS
            ert_prev = [None]
            ec_prev = [None]
            xstage = [None]

            for s in range(PAIRS + 2):
                if 2 <= s + 1 <= PAIRS - 1:
                    load_pair(s + 1)
                for gg in range(NG2 + 1):
                    # p1: r^T matmuls (pair s, group gg)
                    if s < PAIRS and gg < NG2:
                        kq = kq_tiles[s]
                        ps_rt = ps_rt_pool.tile([128, 1024], f32, tag="ps_rt",
                                                name="ps_rt")
                        for j in range(8):
                            nc.tensor.matmul(
                                ps_rt[:, j * 128:(j + 1) * 128],
                                kq[0:64, gg * 1024 + j * 128: gg * 1024 + (j + 1) * 128],
                                landh[s][0:64, 0:128],
                                start=True, stop=True)
                        ert = pw.tile([128, 1024], bf16, tag="ert", name="ert")
                        nc.scalar.activation(ert[:], ps_rt[:], AF.Exp)
                        ert_new = ert
                    # p3: first X^T matmul (pair s-2, group gg-1)
                    if s >= 2 and gg >= 1:
                        pp = s - 2
                        gm = gg - 1
                        if gm % 2 == 0:
                            xstage[0] = pxs.tile([65, 2048], f32, tag="xstage",
                                                 name="xstage")
                        ps_x0 = ps_x_pool.tile([65, 512], f32, tag="ps_x", name="ps_x0")
                        for j in range(4):
                            nc.tensor.matmul(ps_x0[:, j * 128:(j + 1) * 128],
                                             A1[pp][:],
                                             ec_prev[0][:, j * 128:(j + 1) * 128],
                                             start=True, stop=True)
                        nc.vector.tensor_copy(
                            xstage[0][:, (gm % 2) * 1024:(gm % 2) * 1024 + 512],
                            ps_x0[:])
                    # p3: c matmuls + exp (pair s-2, group gg)
                    if s >= 2 and gg < NG2:
                        pp = s - 2
                        kqp = kq_tiles[pp]
                        ps_c = ps_cm_pool.tile([128, 1024], f32, tag="ps_cm",
                                               name="ps_c")
                        for j in range(8):
                            nc.tensor.matmul(
                                ps_c[:, j * 128:(j + 1) * 128],
                                landh[pp][64:128, 128:256],
                                kqp[64:128, gg * 1024 + j * 128: gg * 1024 + (j + 1) * 128],
                                start=True, stop=True)
                        ec = pw.tile([128, 1024], bf16, tag="ec", name="ec")
                        nc.scalar.activation(ec[:], ps_c[:], AF.Exp)
                        ec_new = ec
                    # p1: S accumulation (pair s, group gg-1)
                    if s < PAIRS and gg >= 1:
                        if gg == 1:
                            ps_S = ps_S_pool.tile([128, 65], f32, tag="ps_S",
                                                  name="ps_S")
                            ps_S_handle[s] = ps_S
                        ps_S = ps_S_handle[s]
                        for j in range(8):
                            nc.tensor.matmul(
                                ps_S[:],
                                ert_prev[0][:, j * 128:(j + 1) * 128],
                                v_tiles[s][:, (gg - 1) * 8 + j, :],
                                start=(gg == 1 and j == 0),
                                stop=(gg == NG2 and j == 7),
                                skip_group_check=True)
                    # p3: second X^T matmul + copy + store (pair s-2, group gg-1)
                    if s >= 2 and gg >= 1:
                        pp = s - 2
                        gm = gg - 1
                        ps_x1 = ps_x_pool.tile([65, 512], f32, tag="ps_x", name="ps_x1")
                        for j in range(4):
                            nc.tensor.matmul(ps_x1[:, j * 128:(j + 1) * 128],
                                             A1[pp][:],
                                             ec_prev[0][:, 512 + j * 128: 512 + (j + 1) * 128],
                                             start=True, stop=True)
                        nc.vector.tensor_copy(
                            xstage[0][:, (gm % 2) * 1024 + 512:(gm % 2) * 1024 + 1024],
                            ps_x1[:])
                        if gm % 2 == 1:
                            q = gm // 2
                            nc.sync.dma_start(
                                XOT[pp, :, q * 2048:(q + 1) * 2048], xstage[0][:])
                    if s < PAIRS and gg < NG2:
                        ert_prev[0] = ert_new
                    if s >= 2 and gg < NG2:
                        ec_prev[0] = ec_new
                    # drip-feed two NS stages per gg (emission done by slot 1)
                    for _ in range(2):
                        th = next(ns_iter, None)
                        if th is not None:
                            th()

                # S normalization for pair s (frees ps_S promptly; no NS dep)
                if s < PAIRS:
                    ps_S = ps_S_handle[s]
                    rrec = pw.tile([128, 1], f32, tag="rrec", name="rrec")
                    nc.vector.reciprocal(rrec[:], ps_S[:, 64:65])
                    s_bf = pw.tile([128, 64], bf16, tag="s_bf", name="s_bf")
                    nc.vector.tensor_scalar_mul(s_bf[:], ps_S[:, 0:64], rrec[:])
                    s_bf_handle[s] = s_bf
                # A-matmul for pair s-1 (needs final NS state, emitted by slot 1)
                if 1 <= s <= PAIRS:
                    pa = s - 1
                    vt_fin = ns_state["vt"]
                    ps_A = ps_x_pool.tile([128, 64], f32, tag="ps_x", name="ps_A")
                    nc.tensor.matmul(ps_A[:], vt_fin[:, pa * 128:(pa + 1) * 128],
                                     s_bf_handle[pa][:], start=True, stop=True)
                    a1 = pw.tile([128, 65], bf16, tag="A1", bufs=3, name="a1")
                    nc.vector.memset(a1[:, 64:65], 1.0)
                    nc.vector.tensor_copy(a1[:, 0:64], ps_A[:])
                    A1[pa] = a1

    nc.finalize()
    _cache["nc"] = nc
    return nc


def kernel(Q, K, V, mask):
    from concourse.bass_utils import run_bass_kernel_spmd

    Q = np.asarray(Q, dtype=np.float32)
    K = np.asarray(K, dtype=np.float32)
    V = np.asarray(V, dtype=np.float32)
    BH = B * H
    Qf = Q.reshape(BH, N, D)
    Kf = K.reshape(BH, N, D)
    Vf = V.reshape(BH, N, D)

    # host: top-k selection + global NS init scalar
    landh = np.empty((BH, 128, 256), np.float16)
    land32 = np.empty((BH, 64, 256), np.float32)
    gmax = 0.0
    for i in range(BH):
        sK = Kf[i, :, 0].copy(); sK[0] = np.inf
        iK = np.sort(np.argpartition(-sK, M)[:M])
        sQ = Qf[i, :, 0].copy(); sQ[0] = np.inf
        iQ = np.sort(np.argpartition(-sQ, M)[:M])
        nr = Qf[i, iQ]          # [M, D]
        ncm = Kf[i, iK]         # [M, D]
        land32[i, :, 0:128] = nr.T
        land32[i, :, 128:256] = ncm.T
        landh[i, 0:64, 0:128] = nr.T
        landh[i, 0:64, 128:256] = ncm.T
        landh[i, 64:128, :] = landh[i, 0:64, :]
        md = nr.astype(np.float64) @ ncm.astype(np.float64).T
        e = np.exp(md - md.max(axis=1, keepdims=True))
        k2 = e / e.sum(axis=1, keepdims=True)
        gmax = max(gmax, float(k2.sum(axis=0).max()))

    kqt = np.empty((BH, 128, N), np.float16)
    kqt[:, 0:64, :] = Kf.transpose(0, 2, 1)
    kqt[:, 64:128, :] = Qf.transpose(0, 2, 1)
    v65 = np.empty((BH, N, 65), ml_dtypes.bfloat16)
    v65[:, :, 0:64] = Vf.astype(ml_dtypes.bfloat16)
    v65[:, :, 64] = 1.0
    gs = np.array([[1.0 / gmax]], np.float32)

    nc = _build()
    in_maps = []
    for c in range(NCORES):
        sl = slice(c * PAIRS, (c + 1) * PAIRS)
        in_maps.append({"KQT": kqt[sl], "V65": v65[sl],
                        "LANDH": landh[sl], "LAND32": land32[sl], "GS": gs})
    res = run_bass_kernel_spmd(nc, in_maps, list(range(NCORES)))
    global LAST_RESULTS
    LAST_RESULTS = res
    xot = np.concatenate([res.results[c]["XOT"] for c in range(NCORES)], axis=0)
    X = xot[:, 0:64, :] / xot[:, 64:65, :]
    return np.ascontiguousarray(X.transpose(0, 2, 1)).reshape(B, H, N, D).astype(np.float32)


# revision 17
# speedup vs baseline: 1.2521x; 1.0679x over previous
import numpy as np
import ml_dtypes

B, H, N, D, M = 4, 12, 8192, 64, 128
NCORES = 8
PAIRS = (B * H) // NCORES   # 6 pairs per core
NG2 = 8                     # 8 double-groups of 1024 per pair
SKEW = 3                    # phase-3 runs SKEW slots behind phase-1

_cache = {}


def _build():
    if "nc" in _cache:
        return _cache["nc"]
    import concourse.bacc as bacc
    import concourse.mybir as mybir
    import concourse.tile as tile

    f32 = mybir.dt.float32
    f16 = mybir.dt.float16
    bf16 = mybir.dt.bfloat16
    AF = mybir.ActivationFunctionType

    nc = bacc.Bacc("TRN2", target_bir_lowering=False, debug=False)
    # K^T on partitions 0:64, Q^T on 64:128
    KQT = nc.declare_dram_parameter("KQT", [PAIRS, 128, N], f16, isOutput=False)
    V65 = nc.declare_dram_parameter("V65", [PAIRS, N, 65], bf16, isOutput=False)
    # landmark tiles: cols 0:128 = nr^T, 128:256 = nc^T; rows 64:128 dup rows 0:64
    LANDH = nc.declare_dram_parameter("LANDH", [PAIRS, 128, 256], f16, isOutput=False)
    LAND32 = nc.declare_dram_parameter("LAND32", [PAIRS, 64, 256], f32, isOutput=False)
    GS = nc.declare_dram_parameter("GS", [1, 1], f32, isOutput=False)
    XO = nc.declare_dram_parameter("XO", [PAIRS, N, 64], bf16, isOutput=True)

    with tile.TileContext(nc) as tc:
        with (tc.tile_pool(name="pc", bufs=1) as pc,
              tc.tile_pool(name="pkq", bufs=5) as pkq,
              tc.tile_pool(name="pv", bufs=2) as pv,
              tc.tile_pool(name="pw", bufs=2) as pw,
              tc.tile_pool(name="pxs", bufs=2) as pxs,
              tc.tile_pool(name="pns", bufs=2) as pns,
              tc.tile_pool(name="ps_rt", bufs=1, space="PSUM") as ps_rt_pool,
              tc.tile_pool(name="ps_S", bufs=1, space="PSUM") as ps_S_pool,
              tc.tile_pool(name="ps_cm", bufs=1, space="PSUM") as ps_cm_pool,
              tc.tile_pool(name="ps_x", bufs=1, space="PSUM") as ps_x_pool,
              tc.tile_pool(name="ps_ns", bufs=1, space="PSUM") as ps_ns_pool):

            # ---- bulk loads for pairs 0/1 first (critical path) ----
            kq_tiles = [None] * PAIRS
            v_tiles = [None] * PAIRS

            def load_pair(p):
                kq = pkq.tile([128, N], f16, tag="kq", name="kq")
                nc.gpsimd.dma_start(kq[:], KQT[p])
                vt = pv.tile([128, N // 128, 65], bf16, tag="v65", name="vt")
                nc.gpsimd.dma_start(vt[:], V65[p].rearrange("(t pp) d -> pp t d", pp=128))
                kq_tiles[p] = kq
                v_tiles[p] = vt

            load_pair(0)

            # ---- constants ----
            ident = pc.tile([128, 128], bf16, tag="ident")
            nc.gpsimd.memset(ident[:], 0.0)
            nc.gpsimd.affine_select(out=ident[:], in_=ident[:],
                compare_op=mybir.AluOpType.not_equal, fill=1.0, base=0,
                pattern=[[-1, 128]], channel_multiplier=1)
            diags = {}
            for val, tg in ((7.0, "i7"), (15.0, "i15"), (13.0, "i13")):
                t = pc.tile([128, PAIRS * 128], bf16, tag=tg, name=tg)
                nc.gpsimd.memset(t[:], 0.0)
                for p in range(PAIRS):
                    nc.gpsimd.affine_select(
                        out=t[:, p * 128:(p + 1) * 128],
                        in_=t[:, p * 128:(p + 1) * 128],
                        compare_op=mybir.AluOpType.not_equal, fill=val, base=0,
                        pattern=[[-1, 128]], channel_multiplier=1)
                diags[tg] = t
            i7b, i15b, i13b = diags["i7"], diags["i15"], diags["i13"]

            ones_row = pc.tile([1, 128], f32, tag="ones_row")
            nc.vector.memset(ones_row[:], 1.0)
            gs_sb = pc.tile([1, 1], f32, tag="gs_sb")
            nc.sync.dma_start(gs_sb[:], GS[:])
            ps_bc = ps_cm_pool.tile([128, 1], f32, tag="ps_cm", name="ps_bc")
            nc.tensor.matmul(ps_bc[:], ones_row[:], gs_sb[:], start=True, stop=True)
            gsb = pc.tile([128, 1], f32, tag="gsb")
            nc.scalar.copy(gsb[:], ps_bc[:])

            # ---- landmark loads (small, sync queue) ----
            landh = []
            land32 = []
            for p in range(PAIRS):
                lh = pc.tile([128, 256], f16, tag=f"landh{p}", name="lh")
                l32 = pc.tile([64, 256], f32, tag=f"land32{p}", name="l32")
                nc.sync.dma_start(lh[:], LANDH[p])
                nc.sync.dma_start(l32[:], LAND32[p])
                landh.append(lh)
                land32.append(l32)

            load_pair(1)

            # ---- m-chains: k2 for all pairs into batched tiles ----
            W = PAIRS * 128
            k2_all = pc.tile([128, W], bf16, tag="k2_all")
            for p in range(PAIRS):
                ps_m = ps_cm_pool.tile([128, 128], f32, tag="ps_cm", name="ps_m")
                nc.tensor.matmul(ps_m[:], land32[p][:, 0:128], land32[p][:, 128:256],
                                 start=True, stop=True)
                e_m = pns.tile([128, 128], f32, tag="e_m", name="e_m")
                msum = pns.tile([128, 1], f32, tag="msum", name="msum")
                nc.scalar.activation(e_m[:], ps_m[:], AF.Exp, accum_out=msum[:])
                mrec = pns.tile([128, 1], f32, tag="mrec", name="mrec")
                nc.vector.reciprocal(mrec[:], msum[:])
                nc.vector.tensor_scalar_mul(k2_all[:, p * 128:(p + 1) * 128],
                                            e_m[:], mrec[:])
            ps_t = ps_ns_pool.tile([128, W], bf16, tag="ps_ns", name="ps_t")
            for p in range(PAIRS):
                nc.tensor.transpose(ps_t[:, p * 128:(p + 1) * 128],
                                    k2_all[:, p * 128:(p + 1) * 128], ident[:])
            k2t_all = pc.tile([128, W], bf16, tag="k2t_all")
            nc.scalar.copy(k2t_all[:], ps_t[:])
            vm_all = pns.tile([128, W], bf16, tag="vm", name="vm0")
            nc.vector.tensor_scalar_mul(vm_all[:], ps_t[:], gsb[:])
            vt_all = pns.tile([128, W], bf16, tag="vt", name="vt0")
            nc.vector.tensor_scalar_mul(vt_all[:], k2_all[:], gsb[:])

            ns_state = {"vm": vm_all, "vt": vt_all}

            # ---- batched NS stage thunks ----
            def ns_thunks():
                for _ in range(6):
                    carry = {}

                    def cA():
                        ps_P = ps_ns_pool.tile([128, W], f32, tag="ps_ns", name="ps_P")
                        vm = ns_state["vm"]
                        for p in range(PAIRS):
                            nc.tensor.matmul(ps_P[:, p * 128:(p + 1) * 128],
                                             k2t_all[:, p * 128:(p + 1) * 128],
                                             vm[:, p * 128:(p + 1) * 128],
                                             start=True, stop=True)
                        pbf = pns.tile([128, W], bf16, tag="pbf", name="pbf")
                        nc.scalar.copy(pbf[:], ps_P[:])
                        t1 = pns.tile([128, W], bf16, tag="t1", name="t1")
                        nc.vector.tensor_sub(t1[:], i7b[:], ps_P[:])
                        carry["pbf"] = pbf
                        carry["t1"] = t1

                    def cB():
                        ps_pt = ps_ns_pool.tile([128, W], bf16, tag="ps_ns", name="ps_pt")
                        for p in range(PAIRS):
                            nc.tensor.transpose(ps_pt[:, p * 128:(p + 1) * 128],
                                                carry["pbf"][:, p * 128:(p + 1) * 128],
                                                ident[:])
                        ptb = pns.tile([128, W], bf16, tag="ptb", name="ptb")
                        nc.scalar.copy(ptb[:], ps_pt[:])
                        carry["ptb"] = ptb

                    def cC():
                        ps_u = ps_ns_pool.tile([128, W], f32, tag="ps_ns", name="ps_u")
                        for p in range(PAIRS):
                            nc.tensor.matmul(ps_u[:, p * 128:(p + 1) * 128],
                                             carry["ptb"][:, p * 128:(p + 1) * 128],
                                             carry["t1"][:, p * 128:(p + 1) * 128],
                                             start=True, stop=True)
                        t2 = pns.tile([128, W], bf16, tag="t2", name="t2")
                        nc.vector.tensor_sub(t2[:], i15b[:], ps_u[:])
                        carry["t2"] = t2

                    def cD():
                        ps_w = ps_ns_pool.tile([128, W], f32, tag="ps_ns", name="ps_w")
                        for p in range(PAIRS):
                            nc.tensor.matmul(ps_w[:, p * 128:(p + 1) * 128],
                                             carry["ptb"][:, p * 128:(p + 1) * 128],
                                             carry["t2"][:, p * 128:(p + 1) * 128],
                                             start=True, stop=True)
                        t3 = pns.tile([128, W], bf16, tag="t3", name="t3")
                        nc.vector.tensor_sub(t3[:], i13b[:], ps_w[:])
                        carry["t3"] = t3

                    def cE():
                        vt = ns_state["vt"]
                        ps_v = ps_ns_pool.tile([128, W], f32, tag="ps_ns", name="ps_v")
                        for p in range(PAIRS):
                            nc.tensor.matmul(ps_v[:, p * 128:(p + 1) * 128],
                                             vt[:, p * 128:(p + 1) * 128],
                                             carry["t3"][:, p * 128:(p + 1) * 128],
                                             start=True, stop=True)
                        vm_n = pns.tile([128, W], bf16, tag="vm", name="vm_n")
                        nc.vector.tensor_scalar(vm_n[:], ps_v[:], 0.25, scalar2=None,
                                                op0=mybir.AluOpType.mult)
                        ps_vt = ps_ns_pool.tile([128, W], f32, tag="ps_ns", name="ps_vt")
                        for p in range(PAIRS):
                            nc.tensor.matmul(ps_vt[:, p * 128:(p + 1) * 128],
                                             carry["t3"][:, p * 128:(p + 1) * 128],
                                             vt[:, p * 128:(p + 1) * 128],
                                             start=True, stop=True)
                        vt_n = pns.tile([128, W], bf16, tag="vt", name="vt_n")
                        nc.vector.tensor_scalar(vt_n[:], ps_vt[:], 0.25, scalar2=None,
                                                op0=mybir.AluOpType.mult)
                        ns_state["vm"] = vm_n
                        ns_state["vt"] = vt_n

                    yield cA
                    yield cB
                    yield cC
                    yield cD
                    yield cE

            ns_iter = ns_thunks()
            # kick off first chunks before slot 0 (PE is idle during DMA warmup)
            for _ in range(4):
                th = next(ns_iter, None)
                if th is not None:
                    th()

            # ---- main pipelined slots: p1(s) s<6, p3(s-SKEW) s>=SKEW ----
            A1 = [None] * PAIRS
            ps_S_handle = [None] * PAIRS
            s_bf_handle = [None] * PAIRS
            ert_prev = [None]
            ec_prev = [None]
            xstage = [None]

            for s in range(PAIRS + SKEW):
                if 2 <= s + 1 <= PAIRS - 1:
                    load_pair(s + 1)
                for gg in range(NG2 + 1):
                    # p1: r^T matmuls (pair s, group gg)
                    if s < PAIRS and gg < NG2:
                        kq = kq_tiles[s]
                        ps_rt = ps_rt_pool.tile([128, 1024], f32, tag="ps_rt",
                                                name="ps_rt")
                        for j in range(8):
                            nc.tensor.matmul(
                                ps_rt[:, j * 128:(j + 1) * 128],
                                kq[0:64, gg * 1024 + j * 128: gg * 1024 + (j + 1) * 128],
                                landh[s][0:64, 0:128],
                                start=True, stop=True)
                        ert = pw.tile([128, 1024], bf16, tag="ert", name="ert")
                        nc.scalar.activation(ert[:], ps_rt[:], AF.Exp)
                        ert_new = ert
                    # p3: first half X matmuls + divide (pair s-SKEW, group gg-1)
                    if s >= SKEW and gg >= 1:
                        pp = s - SKEW
                        gm = gg - 1
                        if gm % 2 == 0:
                            xstage[0] = pxs.tile([128, 16, 64], bf16, tag="xstage",
                                                 name="xstage")
                        ps_x0 = ps_x_pool.tile([128, 4, 65], f32, tag="ps_x",
                                               name="ps_x0")
                        for j in range(4):
                            nc.tensor.matmul(ps_x0[:, j, :],
                                             ec_prev[0][:, j * 128:(j + 1) * 128],
                                             A1[pp][:], start=True, stop=True)
                        xr0 = pw.tile([128, 4], f32, tag="xr0", name="xr0")
                        nc.vector.reciprocal(xr0[:], ps_x0[:, :, 64])
                        nc.vector.tensor_tensor(
                            out=xstage[0][:, (gm % 2) * 8:(gm % 2) * 8 + 4, :],
                            in0=ps_x0[:, :, 0:64],
                            in1=xr0.rearrange("p (t o) -> p t o", o=1).to_broadcast([128, 4, 64]),
                            op=mybir.AluOpType.mult)
                    # p3: c matmuls + exp (pair s-SKEW, group gg)
                    if s >= SKEW and gg < NG2:
                        pp = s - SKEW
                        kqp = kq_tiles[pp]
                        ps_c = ps_cm_pool.tile([128, 1024], f32, tag="ps_cm",
                                               name="ps_c")
                        for j in range(8):
                            nc.tensor.matmul(
                                ps_c[:, j * 128:(j + 1) * 128],
                                landh[pp][64:128, 128:256],
                                kqp[64:128, gg * 1024 + j * 128: gg * 1024 + (j + 1) * 128],
                                start=True, stop=True)
                        ec = pw.tile([128, 1024], bf16, tag="ec", name="ec")
                        nc.scalar.activation(ec[:], ps_c[:], AF.Exp)
                        ec_new = ec
                    # p1: S accumulation (pair s, group gg-1)
                    if s < PAIRS and gg >= 1:
                        if gg == 1:
                            ps_S = ps_S_pool.tile([128, 65], f32, tag="ps_S",
                                                  name="ps_S")
                            ps_S_handle[s] = ps_S
                        ps_S = ps_S_handle[s]
                        for j in range(8):
                            nc.tensor.matmul(
                                ps_S[:],
                                ert_prev[0][:, j * 128:(j + 1) * 128],
                                v_tiles[s][:, (gg - 1) * 8 + j, :],
                                start=(gg == 1 and j == 0),
                                stop=(gg == NG2 and j == 7),
                                skip_group_check=True)
                    # p3: second half X matmuls + divide + store
                    if s >= SKEW and gg >= 1:
                        pp = s - SKEW
                        gm = gg - 1
                        ps_x1 = ps_x_pool.tile([128, 4, 65], f32, tag="ps_x",
                                               name="ps_x1")
                        for j in range(4):
                            nc.tensor.matmul(ps_x1[:, j, :],
                                             ec_prev[0][:, 512 + j * 128: 512 + (j + 1) * 128],
                                             A1[pp][:], start=True, stop=True)
                        xr1 = pw.tile([128, 4], f32, tag="xr1", name="xr1")
                        nc.vector.reciprocal(xr1[:], ps_x1[:, :, 64])
                        nc.vector.tensor_tensor(
                            out=xstage[0][:, (gm % 2) * 8 + 4:(gm % 2) * 8 + 8, :],
                            in0=ps_x1[:, :, 0:64],
                            in1=xr1.rearrange("p (t o) -> p t o", o=1).to_broadcast([128, 4, 64]),
                            op=mybir.AluOpType.mult)
                        if gm % 2 == 1:
                            q = gm // 2
                            nc.sync.dma_start(
                                XO[pp, q * 2048:(q + 1) * 2048, :]
                                .rearrange("(t pp2) d -> pp2 t d", pp2=128),
                                xstage[0][:])
                    if s < PAIRS and gg < NG2:
                        ert_prev[0] = ert_new
                    if s >= SKEW and gg < NG2:
                        ec_prev[0] = ec_new
                    # drip-feed one NS stage per gg
                    th = next(ns_iter, None)
                    if th is not None:
                        th()

                # S normalization for pair s (frees ps_S promptly; no NS dep)
                if s < PAIRS:
                    ps_S = ps_S_handle[s]
                    rrec = pw.tile([128, 1], f32, tag="rrec", name="rrec")
                    nc.vector.reciprocal(rrec[:], ps_S[:, 64:65])
                    s_bf = pw.tile([128, 64], bf16, tag="s_bf", name="s_bf")
                    nc.vector.tensor_scalar_mul(s_bf[:], ps_S[:, 0:64], rrec[:])
                    s_bf_handle[s] = s_bf
                # A-matmul for pair s-(SKEW-1): needs final NS state
                if SKEW - 1 <= s < PAIRS + SKEW - 1:
                    pa = s - (SKEW - 1)
                    vt_fin = ns_state["vt"]
                    ps_A = ps_x_pool.tile([128, 64], f32, tag="ps_x", name="ps_A")
                    nc.tensor.matmul(ps_A[:], vt_fin[:, pa * 128:(pa + 1) * 128],
                                     s_bf_handle[pa][:], start=True, stop=True)
                    a1 = pw.tile([128, 65], bf16, tag="A1", bufs=3, name="a1")
                    nc.vector.memset(a1[:, 64:65], 1.0)
                    nc.vector.tensor_copy(a1[:, 0:64], ps_A[:])
                    A1[pa] = a1

    nc.finalize()
    _cache["nc"] = nc
    return nc


def kernel(Q, K, V, mask):
    from concourse.bass_utils import run_bass_kernel_spmd

    Q = np.asarray(Q, dtype=np.float32)
    K = np.asarray(K, dtype=np.float32)
    V = np.asarray(V, dtype=np.float32)
    BH = B * H
    Qf = Q.reshape(BH, N, D)
    Kf = K.reshape(BH, N, D)
    Vf = V.reshape(BH, N, D)

    # host: top-k selection + global NS init scalar
    landh = np.empty((BH, 128, 256), np.float16)
    land32 = np.empty((BH, 64, 256), np.float32)
    gmax = 0.0
    for i in range(BH):
        sK = Kf[i, :, 0].copy(); sK[0] = np.inf
        iK = np.sort(np.argpartition(-sK, M)[:M])
        sQ = Qf[i, :, 0].copy(); sQ[0] = np.inf
        iQ = np.sort(np.argpartition(-sQ, M)[:M])
        nr = Qf[i, iQ]          # [M, D]
        ncm = Kf[i, iK]         # [M, D]
        land32[i, :, 0:128] = nr.T
        land32[i, :, 128:256] = ncm.T
        landh[i, 0:64, 0:128] = nr.T
        landh[i, 0:64, 128:256] = ncm.T
        landh[i, 64:128, :] = landh[i, 0:64, :]
        md = nr.astype(np.float64) @ ncm.astype(np.float64).T
        e = np.exp(md - md.max(axis=1, keepdims=True))
        k2 = e / e.sum(axis=1, keepdims=True)
        gmax = max(gmax, float(k2.sum(axis=0).max()))

    kqt = np.empty((BH, 128, N), np.float16)
    kqt[:, 0:64, :] = Kf.transpose(0, 2, 1)
    kqt[:, 64:128, :] = Qf.transpose(0, 2, 1)
    v65 = np.empty((BH, N, 65), ml_dtypes.bfloat16)
    v65[:, :, 0:64] = Vf.astype(ml_dtypes.bfloat16)
    v65[:, :, 64] = 1.0
    gs = np.array([[1.0 / gmax]], np.float32)

    nc = _build()
    in_maps = []
    for c in range(NCORES):
        sl = slice(c * PAIRS, (c + 1) * PAIRS)
        in_maps.append({"KQT": kqt[sl], "V65": v65[sl],
                        "LANDH": landh[sl], "LAND32": land32[sl], "GS": gs})
    res = run_bass_kernel_spmd(nc, in_maps, list(range(NCORES)))
    global LAST_RESULTS
    LAST_RESULTS = res
    xo = np.concatenate([res.results[c]["XO"] for c in range(NCORES)], axis=0)
    return xo.reshape(B, H, N, D).astype(np.float32)


# revision 19
# speedup vs baseline: 1.2709x; 1.0150x over previous
import numpy as np
import ml_dtypes

B, H, N, D, M = 4, 12, 8192, 64, 128
NCORES = 8
PAIRS = (B * H) // NCORES   # 6 pairs per core
NG2 = 8                     # 8 double-groups of 1024 per pair
SKEW = 3                    # phase-3 runs SKEW slots behind phase-1

_cache = {}


def _build():
    if "nc" in _cache:
        return _cache["nc"]
    import concourse.bacc as bacc
    import concourse.mybir as mybir
    import concourse.tile as tile

    f32 = mybir.dt.float32
    f16 = mybir.dt.float16
    bf16 = mybir.dt.bfloat16
    AF = mybir.ActivationFunctionType

    nc = bacc.Bacc("TRN2", target_bir_lowering=False, debug=False)
    # K^T on partitions 0:64, Q^T on 64:128
    KQT = nc.declare_dram_parameter("KQT", [PAIRS, 128, N], bf16, isOutput=False)
    V65 = nc.declare_dram_parameter("V65", [PAIRS, N, 65], bf16, isOutput=False)
    # landmark tiles: cols 0:128 = nr^T, 128:256 = nc^T; rows 64:128 dup rows 0:64
    LANDH = nc.declare_dram_parameter("LANDH", [128, PAIRS * 256], bf16, isOutput=False)
    LAND32 = nc.declare_dram_parameter("LAND32", [64, PAIRS * 256], f32, isOutput=False)
    GS = nc.declare_dram_parameter("GS", [1, 1], f32, isOutput=False)
    XO = nc.declare_dram_parameter("XO", [PAIRS, N, 64], bf16, isOutput=True)

    with tile.TileContext(nc) as tc:
        with (tc.tile_pool(name="pc", bufs=1) as pc,
              tc.tile_pool(name="pkq", bufs=5) as pkq,
              tc.tile_pool(name="pv", bufs=2) as pv,
              tc.tile_pool(name="pw", bufs=2) as pw,
              tc.tile_pool(name="pxs", bufs=2) as pxs,
              tc.tile_pool(name="pns", bufs=2) as pns,
              tc.tile_pool(name="ps_rt", bufs=1, space="PSUM") as ps_rt_pool,
              tc.tile_pool(name="ps_S", bufs=1, space="PSUM") as ps_S_pool,
              tc.tile_pool(name="ps_cm", bufs=1, space="PSUM") as ps_cm_pool,
              tc.tile_pool(name="ps_x", bufs=1, space="PSUM") as ps_x_pool,
              tc.tile_pool(name="ps_ns", bufs=1, space="PSUM") as ps_ns_pool):

            # ---- landmarks + bulk loads (landmarks first: m-chains/NS
            # prep runs while the big DMAs stream) ----
            gs_sb = pc.tile([1, 1], f32, tag="gs_sb")
            nc.sync.dma_start(gs_sb[:], GS[:])
            landh_all = pc.tile([128, PAIRS * 256], bf16, tag="landh_all")
            nc.sync.dma_start(landh_all[:], LANDH[:])
            land32_all = pc.tile([64, PAIRS * 256], f32, tag="land32_all")
            nc.sync.dma_start(land32_all[:], LAND32[:])
            kq_tiles = [None] * PAIRS
            v_tiles = [None] * PAIRS

            def load_pair(p):
                kq = pkq.tile([128, N], bf16, tag="kq", name="kq")
                nc.gpsimd.dma_start(kq[:], KQT[p])
                vt = pv.tile([128, N // 128, 65], bf16, tag="v65", name="vt")
                nc.gpsimd.dma_start(vt[:], V65[p].rearrange("(t pp) d -> pp t d", pp=128))
                kq_tiles[p] = kq
                v_tiles[p] = vt

            load_pair(0)

            # ---- constants ----
            ident = pc.tile([128, 128], bf16, tag="ident")
            nc.gpsimd.memset(ident[:], 0.0)
            nc.gpsimd.affine_select(out=ident[:], in_=ident[:],
                compare_op=mybir.AluOpType.not_equal, fill=1.0, base=0,
                pattern=[[-1, 128]], channel_multiplier=1)
            diags = {}
            for val, tg in ((7.0, "i7"), (15.0, "i15"), (13.0, "i13")):
                t = pc.tile([128, PAIRS * 128], bf16, tag=tg, name=tg)
                nc.gpsimd.memset(t[:], 0.0)
                for p in range(PAIRS):
                    nc.gpsimd.affine_select(
                        out=t[:, p * 128:(p + 1) * 128],
                        in_=t[:, p * 128:(p + 1) * 128],
                        compare_op=mybir.AluOpType.not_equal, fill=val, base=0,
                        pattern=[[-1, 128]], channel_multiplier=1)
                diags[tg] = t
            i7b, i15b, i13b = diags["i7"], diags["i15"], diags["i13"]

            ones_row = pc.tile([1, 128], f32, tag="ones_row")
            nc.vector.memset(ones_row[:], 1.0)
            ps_bc = ps_cm_pool.tile([128, 1], f32, tag="ps_cm", name="ps_bc")
            nc.tensor.matmul(ps_bc[:], ones_row[:], gs_sb[:], start=True, stop=True)
            gsb = pc.tile([128, 1], f32, tag="gsb")
            nc.scalar.copy(gsb[:], ps_bc[:])

            load_pair(1)

            # ---- m-chains: k2 for all pairs into batched tiles ----
            W = PAIRS * 128
            k2_all = pc.tile([128, W], bf16, tag="k2_all")
            for p in range(PAIRS):
                ps_m = ps_cm_pool.tile([128, 128], f32, tag="ps_cm", name="ps_m")
                nc.tensor.matmul(ps_m[:], land32_all[:, p * 256: p * 256 + 128], land32_all[:, p * 256 + 128: p * 256 + 256],
                                 start=True, stop=True)
                e_m = pns.tile([128, 128], f32, tag="e_m", name="e_m")
                msum = pns.tile([128, 1], f32, tag="msum", name="msum")
                nc.scalar.activation(e_m[:], ps_m[:], AF.Exp, accum_out=msum[:])
                mrec = pns.tile([128, 1], f32, tag="mrec", name="mrec")
                nc.vector.reciprocal(mrec[:], msum[:])
                nc.vector.tensor_scalar_mul(k2_all[:, p * 128:(p + 1) * 128],
                                            e_m[:], mrec[:])
            ps_t = ps_ns_pool.tile([128, W], bf16, tag="ps_ns", name="ps_t")
            for p in range(PAIRS):
                nc.tensor.transpose(ps_t[:, p * 128:(p + 1) * 128],
                                    k2_all[:, p * 128:(p + 1) * 128], ident[:])
            k2t_all = pc.tile([128, W], bf16, tag="k2t_all")
            nc.scalar.copy(k2t_all[:], ps_t[:])
            vm_all = pns.tile([128, W], bf16, tag="vm", name="vm0")
            nc.vector.tensor_scalar_mul(vm_all[:], ps_t[:], gsb[:])
            vt_all = pns.tile([128, W], bf16, tag="vt", name="vt0")
            nc.vector.tensor_scalar_mul(vt_all[:], k2_all[:], gsb[:])

            ns_state = {"vm": vm_all, "vt": vt_all}

            # ---- batched NS stage thunks ----
            def ns_thunks():
                for _ in range(6):
                    carry = {}

                    def cA():
                        ps_P = ps_ns_pool.tile([128, W], f32, tag="ps_ns", name="ps_P")
                        vm = ns_state["vm"]
                        for p in range(PAIRS):
                            nc.tensor.matmul(ps_P[:, p * 128:(p + 1) * 128],
                                             k2t_all[:, p * 128:(p + 1) * 128],
                                             vm[:, p * 128:(p + 1) * 128],
                                             start=True, stop=True)
                        pbf = pns.tile([128, W], bf16, tag="pbf", name="pbf")
                        nc.scalar.copy(pbf[:], ps_P[:])
                        t1 = pns.tile([128, W], bf16, tag="t1", name="t1")
                        nc.vector.tensor_sub(t1[:], i7b[:], ps_P[:])
                        carry["pbf"] = pbf
                        carry["t1"] = t1

                    def cB():
                        ps_pt = ps_ns_pool.tile([128, W], bf16, tag="ps_ns", name="ps_pt")
                        for p in range(PAIRS):
                            nc.tensor.transpose(ps_pt[:, p * 128:(p + 1) * 128],
                                                carry["pbf"][:, p * 128:(p + 1) * 128],
                                                ident[:])
                        ptb = pns.tile([128, W], bf16, tag="ptb", name="ptb")
                        nc.scalar.copy(ptb[:], ps_pt[:])
                        carry["ptb"] = ptb

                    def cC():
                        ps_u = ps_ns_pool.tile([128, W], f32, tag="ps_ns", name="ps_u")
                        for p in range(PAIRS):
                            nc.tensor.matmul(ps_u[:, p * 128:(p + 1) * 128],
                                             carry["ptb"][:, p * 128:(p + 1) * 128],
                                             carry["t1"][:, p * 128:(p + 1) * 128],
                                             start=True, stop=True)
                        t2 = pns.tile([128, W], bf16, tag="t2", name="t2")
                        nc.vector.tensor_sub(t2[:], i15b[:], ps_u[:])
                        carry["t2"] = t2

                    def cD():
                        ps_w = ps_ns_pool.tile([128, W], f32, tag="ps_ns", name="ps_w")
                        for p in range(PAIRS):
                            nc.tensor.matmul(ps_w[:, p * 128:(p + 1) * 128],
                                             carry["ptb"][:, p * 128:(p + 1) * 128],
                                             carry["t2"][:, p * 128:(p + 1) * 128],
                                             start=True, stop=True)
                        t3 = pns.tile([128, W], bf16, tag="t3", name="t3")
                        nc.vector.tensor_sub(t3[:], i13b[:], ps_w[:])
                        carry["t3"] = t3

                    def cE():
                        vt = ns_state["vt"]
                        ps_v = ps_ns_pool.tile([128, W], f32, tag="ps_ns", name="ps_v")
                        for p in range(PAIRS):
                            nc.tensor.matmul(ps_v[:, p * 128:(p + 1) * 128],
                                             vt[:, p * 128:(p + 1) * 128],
                                             carry["t3"][:, p * 128:(p + 1) * 128],
                                             start=True, stop=True)
                        vm_n = pns.tile([128, W], bf16, tag="vm", name="vm_n")
                        nc.vector.tensor_scalar(vm_n[:], ps_v[:], 0.25, scalar2=None,
                                                op0=mybir.AluOpType.mult)
                        ps_vt = ps_ns_pool.tile([128, W], f32, tag="ps_ns", name="ps_vt")
                        for p in range(PAIRS):
                            nc.tensor.matmul(ps_vt[:, p * 128:(p + 1) * 128],
                                             carry["t3"][:, p * 128:(p + 1) * 128],
                                             vt[:, p * 128:(p + 1) * 128],
                                             start=True, stop=True)
                        vt_n = pns.tile([128, W], bf16, tag="vt", name="vt_n")
                        nc.vector.tensor_scalar(vt_n[:], ps_vt[:], 0.25, scalar2=None,
                                                op0=mybir.AluOpType.mult)
                        ns_state["vm"] = vm_n
                        ns_state["vt"] = vt_n

                    yield cA
                    yield cB
                    yield cC
                    yield cD
                    yield cE

            ns_iter = ns_thunks()
            # kick off first chunks before slot 0 (PE is idle during DMA warmup)
            for _ in range(4):
                th = next(ns_iter, None)
                if th is not None:
                    th()

            # ---- main pipelined slots: p1(s) s<6, p3(s-SKEW) s>=SKEW ----
            A1 = [None] * PAIRS
            ps_S_handle = [None] * PAIRS
            s_bf_handle = [None] * PAIRS
            ert_prev = [None]
            ec_prev = [None]
            xstage = [None]

            for s in range(PAIRS + SKEW):
                if 2 <= s + 1 <= PAIRS - 1:
                    load_pair(s + 1)
                for gg in range(NG2 + 1):
                    # p1: r^T matmuls (pair s, group gg)
                    if s < PAIRS and gg < NG2:
                        kq = kq_tiles[s]
                        ps_rt = ps_rt_pool.tile([128, 1024], f32, tag="ps_rt",
                                                name="ps_rt")
                        for j in range(8):
                            nc.tensor.matmul(
                                ps_rt[:, j * 128:(j + 1) * 128],
                                kq[0:64, gg * 1024 + j * 128: gg * 1024 + (j + 1) * 128],
                                landh_all[0:64, s * 256: s * 256 + 128],
                                start=True, stop=True)
                        ert = pw.tile([128, 1024], bf16, tag="ert", name="ert")
                        nc.scalar.activation(ert[:], ps_rt[:], AF.Exp)
                        ert_new = ert
                    # p3: first half X matmuls + divide (pair s-SKEW, group gg-1)
                    if s >= SKEW and gg >= 1:
                        pp = s - SKEW
                        gm = gg - 1
                        if gm % 2 == 0:
                            xstage[0] = pxs.tile([128, 16, 64], bf16, tag="xstage",
                                                 name="xstage")
                        ps_x0 = ps_x_pool.tile([128, 4, 65], f32, tag="ps_x",
                                               name="ps_x0")
                        for j in range(4):
                            nc.tensor.matmul(ps_x0[:, j, :],
                                             ec_prev[0][:, j * 128:(j + 1) * 128],
                                             A1[pp][:], start=True, stop=True)
                        xr0 = pw.tile([128, 4], f32, tag="xr0", name="xr0")
                        nc.vector.reciprocal(xr0[:], ps_x0[:, :, 64])
                        nc.vector.tensor_tensor(
                            out=xstage[0][:, (gm % 2) * 8:(gm % 2) * 8 + 4, :],
                            in0=ps_x0[:, :, 0:64],
                            in1=xr0.rearrange("p (t o) -> p t o", o=1).to_broadcast([128, 4, 64]),
                            op=mybir.AluOpType.mult)
                    # p3: c matmuls + exp (pair s-SKEW, group gg)
                    if s >= SKEW and gg < NG2:
                        pp = s - SKEW
                        kqp = kq_tiles[pp]
                        ps_c = ps_cm_pool.tile([128, 1024], f32, tag="ps_cm",
                                               name="ps_c")
                        for j in range(8):
                            nc.tensor.matmul(
                                ps_c[:, j * 128:(j + 1) * 128],
                                landh_all[64:128, pp * 256 + 128: pp * 256 + 256],
                                kqp[64:128, gg * 1024 + j * 128: gg * 1024 + (j + 1) * 128],
                                start=True, stop=True)
                        ec = pw.tile([128, 1024], bf16, tag="ec", name="ec")
                        nc.scalar.activation(ec[:], ps_c[:], AF.Exp)
                        ec_new = ec
                    # p1: S accumulation (pair s, group gg-1)
                    if s < PAIRS and gg >= 1:
                        if gg == 1:
                            ps_S = ps_S_pool.tile([128, 65], f32, tag="ps_S",
                                                  name="ps_S")
                            ps_S_handle[s] = ps_S
                        ps_S = ps_S_handle[s]
                        for j in range(8):
                            nc.tensor.matmul(
                                ps_S[:],
                                ert_prev[0][:, j * 128:(j + 1) * 128],
                                v_tiles[s][:, (gg - 1) * 8 + j, :],
                                start=(gg == 1 and j == 0),
                                stop=(gg == NG2 and j == 7),
                                skip_group_check=True)
                    # p3: second half X matmuls + divide + store
                    if s >= SKEW and gg >= 1:
                        pp = s - SKEW
                        gm = gg - 1
                        ps_x1 = ps_x_pool.tile([128, 4, 65], f32, tag="ps_x",
                                               name="ps_x1")
                        for j in range(4):
                            nc.tensor.matmul(ps_x1[:, j, :],
                                             ec_prev[0][:, 512 + j * 128: 512 + (j + 1) * 128],
                                             A1[pp][:], start=True, stop=True)
                        xr1 = pw.tile([128, 4], f32, tag="xr1", name="xr1")
                        nc.vector.reciprocal(xr1[:], ps_x1[:, :, 64])
                        nc.vector.tensor_tensor(
                            out=xstage[0][:, (gm % 2) * 8 + 4:(gm % 2) * 8 + 8, :],
                            in0=ps_x1[:, :, 0:64],
                            in1=xr1.rearrange("p (t o) -> p t o", o=1).to_broadcast([128, 4, 64]),
                            op=mybir.AluOpType.mult)
                        if gm % 2 == 1:
                            q = gm // 2
                            nc.sync.dma_start(
                                XO[pp, q * 2048:(q + 1) * 2048, :]
                                .rearrange("(t pp2) d -> pp2 t d", pp2=128),
                                xstage[0][:])
                    if s < PAIRS and gg < NG2:
                        ert_prev[0] = ert_new
                    if s >= SKEW and gg < NG2:
                        ec_prev[0] = ec_new
                    # drip-feed one NS stage per gg
                    th = next(ns_iter, None)
                    if th is not None:
                        th()

                # S normalization for pair s (frees ps_S promptly; no NS dep)
                if s < PAIRS:
                    ps_S = ps_S_handle[s]
                    rrec = pw.tile([128, 1], f32, tag="rrec", name="rrec")
                    nc.vector.reciprocal(rrec[:], ps_S[:, 64:65])
                    s_bf = pw.tile([128, 64], bf16, tag="s_bf", name="s_bf")
                    nc.vector.tensor_scalar_mul(s_bf[:], ps_S[:, 0:64], rrec[:])
                    s_bf_handle[s] = s_bf
                # A-matmul for pair s-(SKEW-1): needs final NS state
                if SKEW - 1 <= s < PAIRS + SKEW - 1:
                    pa = s - (SKEW - 1)
                    vt_fin = ns_state["vt"]
                    ps_A = ps_x_pool.tile([128, 64], f32, tag="ps_x", name="ps_A")
                    nc.tensor.matmul(ps_A[:], vt_fin[:, pa * 128:(pa + 1) * 128],
                                     s_bf_handle[pa][:], start=True, stop=True)
                    a1 = pw.tile([128, 65], bf16, tag="A1", bufs=3, name="a1")
                    nc.vector.memset(a1[:, 64:65], 1.0)
                    nc.vector.tensor_copy(a1[:, 0:64], ps_A[:])
                    A1[pa] = a1

    nc.finalize()
    _cache["nc"] = nc
    return nc


def kernel(Q, K, V, mask):
    from concourse.bass_utils import run_bass_kernel_spmd

    Q = np.asarray(Q, dtype=np.float32)
    K = np.asarray(K, dtype=np.float32)
    V = np.asarray(V, dtype=np.float32)
    BH = B * H
    Qf = Q.reshape(BH, N, D)
    Kf = K.reshape(BH, N, D)
    Vf = V.reshape(BH, N, D)

    # host: top-k selection + global NS init scalar
    landh = np.empty((BH, 128, 256), ml_dtypes.bfloat16)
    land32 = np.empty((BH, 64, 256), np.float32)
    gmax = 0.0
    for i in range(BH):
        sK = Kf[i, :, 0].copy(); sK[0] = np.inf
        iK = np.sort(np.argpartition(-sK, M)[:M])
        sQ = Qf[i, :, 0].copy(); sQ[0] = np.inf
        iQ = np.sort(np.argpartition(-sQ, M)[:M])
        nr = Qf[i, iQ]          # [M, D]
        ncm = Kf[i, iK]         # [M, D]
        land32[i, :, 0:128] = nr.T
        land32[i, :, 128:256] = ncm.T
        landh[i, 0:64, 0:128] = nr.T.astype(ml_dtypes.bfloat16)
        landh[i, 0:64, 128:256] = ncm.T.astype(ml_dtypes.bfloat16)
        landh[i, 64:128, :] = landh[i, 0:64, :]
        md = nr.astype(np.float64) @ ncm.astype(np.float64).T
        e = np.exp(md - md.max(axis=1, keepdims=True))
        k2 = e / e.sum(axis=1, keepdims=True)
        gmax = max(gmax, float(k2.sum(axis=0).max()))

    kqt = np.empty((BH, 128, N), ml_dtypes.bfloat16)
    kqt[:, 0:64, :] = Kf.transpose(0, 2, 1).astype(ml_dtypes.bfloat16)
    kqt[:, 64:128, :] = Qf.transpose(0, 2, 1).astype(ml_dtypes.bfloat16)
    v65 = np.empty((BH, N, 65), ml_dtypes.bfloat16)
    v65[:, :, 0:64] = Vf.astype(ml_dtypes.bfloat16)
    v65[:, :, 64] = 1.0
    gs = np.array([[1.0 / gmax]], np.float32)

    nc = _build()
    in_maps = []
    for c in range(NCORES):
        sl = slice(c * PAIRS, (c + 1) * PAIRS)
        lh = np.ascontiguousarray(
            landh[sl].transpose(1, 0, 2).reshape(128, PAIRS * 256))
        l32 = np.ascontiguousarray(
            land32[sl].transpose(1, 0, 2).reshape(64, PAIRS * 256))
        in_maps.append({"KQT": kqt[sl], "V65": v65[sl],
                        "LANDH": lh, "LAND32": l32, "GS": gs})
    res = run_bass_kernel_spmd(nc, in_maps, list(range(NCORES)))
    global LAST_RESULTS
    LAST_RESULTS = res
    xo = np.concatenate([res.results[c]["XO"] for c in range(NCORES)], axis=0)
    return xo.reshape(B, H, N, D).astype(np.float32)


# revision 20
# speedup vs baseline: 1.3223x; 1.0404x over previous
import numpy as np
import ml_dtypes

B, H, N, D, M = 4, 12, 8192, 64, 128
NCORES = 8
PAIRS = (B * H) // NCORES   # 6 pairs per core
NG2 = 8                     # 8 double-groups of 1024 per pair
SKEW = 3                    # phase-3 runs SKEW slots behind phase-1

_cache = {}


def _build():
    if "nc" in _cache:
        return _cache["nc"]
    import concourse.bacc as bacc
    import concourse.mybir as mybir
    import concourse.tile as tile

    f32 = mybir.dt.float32
    f16 = mybir.dt.float16
    bf16 = mybir.dt.bfloat16
    AF = mybir.ActivationFunctionType

    nc = bacc.Bacc("TRN2", target_bir_lowering=False, debug=False)
    # K^T on partitions 0:64, Q^T on 64:128
    KQT = nc.declare_dram_parameter("KQT", [PAIRS, 128, N], f16, isOutput=False)
    V65 = nc.declare_dram_parameter("V65", [PAIRS, 128, N // 128, 65], bf16, isOutput=False)
    # landmark tiles: cols 0:128 = nr^T, 128:256 = nc^T; rows 64:128 dup rows 0:64
    LANDH = nc.declare_dram_parameter("LANDH", [128, PAIRS * 256], f16, isOutput=False)
    LAND32 = nc.declare_dram_parameter("LAND32", [64, PAIRS * 256], f32, isOutput=False)
    GS = nc.declare_dram_parameter("GS", [1, 1], f32, isOutput=False)
    XO = nc.declare_dram_parameter("XO", [PAIRS, 128, N // 128, 64], bf16, isOutput=True)

    with tile.TileContext(nc) as tc:
        with (tc.tile_pool(name="pc", bufs=1) as pc,
              tc.tile_pool(name="pkq", bufs=5) as pkq,
              tc.tile_pool(name="pv", bufs=2) as pv,
              tc.tile_pool(name="pw", bufs=2) as pw,
              tc.tile_pool(name="pxs", bufs=2) as pxs,
              tc.tile_pool(name="pns", bufs=2) as pns,
              tc.tile_pool(name="ps_rt", bufs=1, space="PSUM") as ps_rt_pool,
              tc.tile_pool(name="ps_S", bufs=1, space="PSUM") as ps_S_pool,
              tc.tile_pool(name="ps_cm", bufs=1, space="PSUM") as ps_cm_pool,
              tc.tile_pool(name="ps_x", bufs=1, space="PSUM") as ps_x_pool,
              tc.tile_pool(name="ps_ns", bufs=1, space="PSUM") as ps_ns_pool):

            # ---- landmarks + bulk loads (landmarks first: m-chains/NS
            # prep runs while the big DMAs stream) ----
            gs_sb = pc.tile([1, 1], f32, tag="gs_sb")
            nc.sync.dma_start(gs_sb[:], GS[:])
            landh_all = pc.tile([128, PAIRS * 256], f16, tag="landh_all")
            nc.sync.dma_start(landh_all[:], LANDH[:])
            land32_all = pc.tile([64, PAIRS * 256], f32, tag="land32_all")
            nc.sync.dma_start(land32_all[:], LAND32[:])
            kq_tiles = [None] * PAIRS
            v_tiles = [None] * PAIRS

            def load_pair(p):
                kq = pkq.tile([128, N], f16, tag="kq", name="kq")
                nc.gpsimd.dma_start(kq[:], KQT[p])
                vt = pv.tile([128, N // 128, 65], bf16, tag="v65", name="vt")
                nc.gpsimd.dma_start(vt[:], V65[p])
                kq_tiles[p] = kq
                v_tiles[p] = vt

            load_pair(0)

            # ---- constants ----
            ident = pc.tile([128, 128], bf16, tag="ident")
            nc.gpsimd.memset(ident[:], 0.0)
            nc.gpsimd.affine_select(out=ident[:], in_=ident[:],
                compare_op=mybir.AluOpType.not_equal, fill=1.0, base=0,
                pattern=[[-1, 128]], channel_multiplier=1)
            diags = {}
            for val, tg in ((7.0, "i7"), (15.0, "i15"), (13.0, "i13")):
                t = pc.tile([128, PAIRS * 128], bf16, tag=tg, name=tg)
                nc.gpsimd.memset(t[:], 0.0)
                for p in range(PAIRS):
                    nc.gpsimd.affine_select(
                        out=t[:, p * 128:(p + 1) * 128],
                        in_=t[:, p * 128:(p + 1) * 128],
                        compare_op=mybir.AluOpType.not_equal, fill=val, base=0,
                        pattern=[[-1, 128]], channel_multiplier=1)
                diags[tg] = t
            i7b, i15b, i13b = diags["i7"], diags["i15"], diags["i13"]

            ones_row = pc.tile([1, 128], f32, tag="ones_row")
            nc.vector.memset(ones_row[:], 1.0)
            ps_bc = ps_cm_pool.tile([128, 1], f32, tag="ps_cm", name="ps_bc")
            nc.tensor.matmul(ps_bc[:], ones_row[:], gs_sb[:], start=True, stop=True)
            gsb = pc.tile([128, 1], f32, tag="gsb")
            nc.scalar.copy(gsb[:], ps_bc[:])

            load_pair(1)

            # ---- m-chains: k2 for all pairs into batched tiles ----
            W = PAIRS * 128
            k2_all = pc.tile([128, W], bf16, tag="k2_all")
            for p in range(PAIRS):
                ps_m = ps_cm_pool.tile([128, 128], f32, tag="ps_cm", name="ps_m")
                nc.tensor.matmul(ps_m[:], land32_all[:, p * 256: p * 256 + 128], land32_all[:, p * 256 + 128: p * 256 + 256],
                                 start=True, stop=True)
                e_m = pns.tile([128, 128], f32, tag="e_m", name="e_m")
                msum = pns.tile([128, 1], f32, tag="msum", name="msum")
                nc.scalar.activation(e_m[:], ps_m[:], AF.Exp, accum_out=msum[:])
                mrec = pns.tile([128, 1], f32, tag="mrec", name="mrec")
                nc.vector.reciprocal(mrec[:], msum[:])
                nc.vector.tensor_scalar_mul(k2_all[:, p * 128:(p + 1) * 128],
                                            e_m[:], mrec[:])
            ps_t = ps_ns_pool.tile([128, W], bf16, tag="ps_ns", name="ps_t")
            for p in range(PAIRS):
                nc.tensor.transpose(ps_t[:, p * 128:(p + 1) * 128],
                                    k2_all[:, p * 128:(p + 1) * 128], ident[:])
            k2t_all = pc.tile([128, W], bf16, tag="k2t_all")
            nc.scalar.copy(k2t_all[:], ps_t[:])
            vm_all = pns.tile([128, W], bf16, tag="vm", name="vm0")
            nc.vector.tensor_scalar_mul(vm_all[:], ps_t[:], gsb[:])
            vt_all = pns.tile([128, W], bf16, tag="vt", name="vt0")
            nc.vector.tensor_scalar_mul(vt_all[:], k2_all[:], gsb[:])

            ns_state = {"vm": vm_all, "vt": vt_all}

            # ---- batched NS stage thunks ----
            def ns_thunks():
                for _ in range(6):
                    carry = {}

                    def cA():
                        ps_P = ps_ns_pool.tile([128, W], f32, tag="ps_ns", name="ps_P")
                        vm = ns_state["vm"]
                        for p in range(PAIRS):
                            nc.tensor.matmul(ps_P[:, p * 128:(p + 1) * 128],
                                             k2t_all[:, p * 128:(p + 1) * 128],
                                             vm[:, p * 128:(p + 1) * 128],
                                             start=True, stop=True)
                        pbf = pns.tile([128, W], bf16, tag="pbf", name="pbf")
                        nc.scalar.copy(pbf[:], ps_P[:])
                        t1 = pns.tile([128, W], bf16, tag="t1", name="t1")
                        nc.vector.tensor_sub(t1[:], i7b[:], ps_P[:])
                        carry["pbf"] = pbf
                        carry["t1"] = t1

                    def cB():
                        ps_pt = ps_ns_pool.tile([128, W], bf16, tag="ps_ns", name="ps_pt")
                        for p in range(PAIRS):
                            nc.tensor.transpose(ps_pt[:, p * 128:(p + 1) * 128],
                                                carry["pbf"][:, p * 128:(p + 1) * 128],
                                                ident[:])
                        ptb = pns.tile([128, W], bf16, tag="ptb", name="ptb")
                        nc.scalar.copy(ptb[:], ps_pt[:])
                        carry["ptb"] = ptb

                    def cC():
                        ps_u = ps_ns_pool.tile([128, W], f32, tag="ps_ns", name="ps_u")
                        for p in range(PAIRS):
                            nc.tensor.matmul(ps_u[:, p * 128:(p + 1) * 128],
                                             carry["ptb"][:, p * 128:(p + 1) * 128],
                                             carry["t1"][:, p * 128:(p + 1) * 128],
                                             start=True, stop=True)
                        t2 = pns.tile([128, W], bf16, tag="t2", name="t2")
                        nc.vector.tensor_sub(t2[:], i15b[:], ps_u[:])
                        carry["t2"] = t2

                    def cD():
                        ps_w = ps_ns_pool.tile([128, W], f32, tag="ps_ns", name="ps_w")
                        for p in range(PAIRS):
                            nc.tensor.matmul(ps_w[:, p * 128:(p + 1) * 128],
                                             carry["ptb"][:, p * 128:(p + 1) * 128],
                                             carry["t2"][:, p * 128:(p + 1) * 128],
                                             start=True, stop=True)
                        t3 = pns.tile([128, W], bf16, tag="t3", name="t3")
                        nc.vector.tensor_sub(t3[:], i13b[:], ps_w[:])
                        carry["t3"] = t3

                    def cE():
                        vt = ns_state["vt"]
                        ps_v = ps_ns_pool.tile([128, W], f32, tag="ps_ns", name="ps_v")
                        for p in range(PAIRS):
                            nc.tensor.matmul(ps_v[:, p * 128:(p + 1) * 128],
                                             vt[:, p * 128:(p + 1) * 128],
                                             carry["t3"][:, p * 128:(p + 1) * 128],
                                             start=True, stop=True)
                        vm_n = pns.tile([128, W], bf16, tag="vm", name="vm_n")
                        nc.vector.tensor_scalar(vm_n[:], ps_v[:], 0.25, scalar2=None,
                                                op0=mybir.AluOpType.mult)
                        ps_vt = ps_ns_pool.tile([128, W], f32, tag="ps_ns", name="ps_vt")
                        for p in range(PAIRS):
                            nc.tensor.matmul(ps_vt[:, p * 128:(p + 1) * 128],
                                             carry["t3"][:, p * 128:(p + 1) * 128],
                                             vt[:, p * 128:(p + 1) * 128],
                                             start=True, stop=True)
                        vt_n = pns.tile([128, W], bf16, tag="vt", name="vt_n")
                        nc.vector.tensor_scalar(vt_n[:], ps_vt[:], 0.25, scalar2=None,
                                                op0=mybir.AluOpType.mult)
                        ns_state["vm"] = vm_n
                        ns_state["vt"] = vt_n

                    yield cA
                    yield cB
                    yield cC
                    yield cD
                    yield cE

            ns_iter = ns_thunks()
            # kick off first chunks before slot 0 (PE is idle during DMA warmup)
            for _ in range(4):
                th = next(ns_iter, None)
                if th is not None:
                    th()

            # ---- main pipelined slots: p1(s) s<6, p3(s-SKEW) s>=SKEW ----
            A1 = [None] * PAIRS
            ps_S_handle = [None] * PAIRS
            s_bf_handle = [None] * PAIRS
            ert_prev = [None]
            ec_prev = [None]
            xstage = [None]

            for s in range(PAIRS + SKEW):
                if 2 <= s + 1 <= PAIRS - 1:
                    load_pair(s + 1)
                for gg in range(NG2 + 1):
                    # p1: r^T matmuls (pair s, group gg)
                    if s < PAIRS and gg < NG2:
                        kq = kq_tiles[s]
                        ps_rt = ps_rt_pool.tile([128, 1024], f32, tag="ps_rt",
                                                name="ps_rt")
                        for j in range(8):
                            nc.tensor.matmul(
                                ps_rt[:, j * 128:(j + 1) * 128],
                                kq[0:64, gg * 1024 + j * 128: gg * 1024 + (j + 1) * 128],
                                landh_all[0:64, s * 256: s * 256 + 128],
                                start=True, stop=True)
                        ert = pw.tile([128, 1024], bf16, tag="ert", name="ert")
                        nc.scalar.activation(ert[:], ps_rt[:], AF.Exp)
                        ert_new = ert
                    # p3: first half X matmuls + divide (pair s-SKEW, group gg-1)
                    if s >= SKEW and gg >= 1:
                        pp = s - SKEW
                        gm = gg - 1
                        if gm % 2 == 0:
                            xstage[0] = pxs.tile([128, 16, 64], bf16, tag="xstage",
                                                 name="xstage")
                        ps_x0 = ps_x_pool.tile([128, 4, 65], f32, tag="ps_x",
                                               name="ps_x0")
                        for j in range(4):
                            nc.tensor.matmul(ps_x0[:, j, :],
                                             ec_prev[0][:, j * 128:(j + 1) * 128],
                                             A1[pp][:], start=True, stop=True)
                        xr0 = pw.tile([128, 4], f32, tag="xr0", name="xr0")
                        nc.vector.reciprocal(xr0[:], ps_x0[:, :, 64])
                        nc.vector.tensor_tensor(
                            out=xstage[0][:, (gm % 2) * 8:(gm % 2) * 8 + 4, :],
                            in0=ps_x0[:, :, 0:64],
                            in1=xr0.rearrange("p (t o) -> p t o", o=1).to_broadcast([128, 4, 64]),
                            op=mybir.AluOpType.mult)
                    # p3: c matmuls + exp (pair s-SKEW, group gg)
                    if s >= SKEW and gg < NG2:
                        pp = s - SKEW
                        kqp = kq_tiles[pp]
                        ps_c = ps_cm_pool.tile([128, 1024], f32, tag="ps_cm",
                                               name="ps_c")
                        for j in range(8):
                            nc.tensor.matmul(
                                ps_c[:, j * 128:(j + 1) * 128],
                                landh_all[64:128, pp * 256 + 128: pp * 256 + 256],
                                kqp[64:128, gg * 1024 + j * 128: gg * 1024 + (j + 1) * 128],
                                start=True, stop=True)
                        ec = pw.tile([128, 1024], bf16, tag="ec", name="ec")
                        nc.scalar.activation(ec[:], ps_c[:], AF.Exp)
                        ec_new = ec
                    # p1: S accumulation (pair s, group gg-1)
                    if s < PAIRS and gg >= 1:
                        if gg == 1:
                            ps_S = ps_S_pool.tile([128, 65], f32, tag="ps_S",
                                                  name="ps_S")
                            ps_S_handle[s] = ps_S
                        ps_S = ps_S_handle[s]
                        for j in range(8):
                            nc.tensor.matmul(
                                ps_S[:],
                                ert_prev[0][:, j * 128:(j + 1) * 128],
                                v_tiles[s][:, (gg - 1) * 8 + j, :],
                                start=(gg == 1 and j == 0),
                                stop=(gg == NG2 and j == 7),
                                skip_group_check=True)
                    # p3: second half X matmuls + divide + store
                    if s >= SKEW and gg >= 1:
                        pp = s - SKEW
                        gm = gg - 1
                        ps_x1 = ps_x_pool.tile([128, 4, 65], f32, tag="ps_x",
                                               name="ps_x1")
                        for j in range(4):
                            nc.tensor.matmul(ps_x1[:, j, :],
                                             ec_prev[0][:, 512 + j * 128: 512 + (j + 1) * 128],
                                             A1[pp][:], start=True, stop=True)
                        xr1 = pw.tile([128, 4], f32, tag="xr1", name="xr1")
                        nc.vector.reciprocal(xr1[:], ps_x1[:, :, 64])
                        nc.vector.tensor_tensor(
                            out=xstage[0][:, (gm % 2) * 8 + 4:(gm % 2) * 8 + 8, :],
                            in0=ps_x1[:, :, 0:64],
                            in1=xr1.rearrange("p (t o) -> p t o", o=1).to_broadcast([128, 4, 64]),
                            op=mybir.AluOpType.mult)
                        if gm % 2 == 1:
                            q = gm // 2
                            nc.sync.dma_start(
                                XO[pp, :, q * 16:(q + 1) * 16, :], xstage[0][:])
                    if s < PAIRS and gg < NG2:
                        ert_prev[0] = ert_new
                    if s >= SKEW and gg < NG2:
                        ec_prev[0] = ec_new
                    # drip-feed one NS stage per gg
                    th = next(ns_iter, None)
                    if th is not None:
                        th()

                # S normalization for pair s (frees ps_S promptly; no NS dep)
                if s < PAIRS:
                    ps_S = ps_S_handle[s]
                    rrec = pw.tile([128, 1], f32, tag="rrec", name="rrec")
                    nc.vector.reciprocal(rrec[:], ps_S[:, 64:65])
                    s_bf = pw.tile([128, 64], bf16, tag="s_bf", name="s_bf")
                    nc.vector.tensor_scalar_mul(s_bf[:], ps_S[:, 0:64], rrec[:])
                    s_bf_handle[s] = s_bf
                # A-matmul for pair s-(SKEW-1): needs final NS state
                if SKEW - 1 <= s < PAIRS + SKEW - 1:
                    pa = s - (SKEW - 1)
                    vt_fin = ns_state["vt"]
                    ps_A = ps_x_pool.tile([128, 64], f32, tag="ps_x", name="ps_A")
                    nc.tensor.matmul(ps_A[:], vt_fin[:, pa * 128:(pa + 1) * 128],
                                     s_bf_handle[pa][:], start=True, stop=True)
                    a1 = pw.tile([128, 65], bf16, tag="A1", bufs=3, name="a1")
                    nc.vector.memset(a1[:, 64:65], 1.0)
                    nc.vector.tensor_copy(a1[:, 0:64], ps_A[:])
                    A1[pa] = a1

    nc.finalize()
    _cache["nc"] = nc
    return nc


def kernel(Q, K, V, mask):
    from concourse.bass_utils import run_bass_kernel_spmd

    Q = np.asarray(Q, dtype=np.float32)
    K = np.asarray(K, dtype=np.float32)
    V = np.asarray(V, dtype=np.float32)
    BH = B * H
    Qf = Q.reshape(BH, N, D)
    Kf = K.reshape(BH, N, D)
    Vf = V.reshape(BH, N, D)

    # host: top-k selection + global NS init scalar
    landh = np.empty((BH, 128, 256), np.float16)
    land32 = np.empty((BH, 64, 256), np.float32)
    gmax = 0.0
    for i in range(BH):
        sK = Kf[i, :, 0].copy(); sK[0] = np.inf
        iK = np.sort(np.argpartition(-sK, M)[:M])
        sQ = Qf[i, :, 0].copy(); sQ[0] = np.inf
        iQ = np.sort(np.argpartition(-sQ, M)[:M])
        nr = Qf[i, iQ]          # [M, D]
        ncm = Kf[i, iK]         # [M, D]
        land32[i, :, 0:128] = nr.T
        land32[i, :, 128:256] = ncm.T
        landh[i, 0:64, 0:128] = nr.T
        landh[i, 0:64, 128:256] = ncm.T
        landh[i, 64:128, :] = landh[i, 0:64, :]
        md = nr.astype(np.float64) @ ncm.astype(np.float64).T
        e = np.exp(md - md.max(axis=1, keepdims=True))
        k2 = e / e.sum(axis=1, keepdims=True)
        gmax = max(gmax, float(k2.sum(axis=0).max()))

    kqt = np.empty((BH, 128, N), np.float16)
    kqt[:, 0:64, :] = Kf.transpose(0, 2, 1)
    kqt[:, 64:128, :] = Qf.transpose(0, 2, 1)
    v65 = np.empty((BH, N, 65), ml_dtypes.bfloat16)
    v65[:, :, 0:64] = Vf.astype(ml_dtypes.bfloat16)
    v65[:, :, 64] = 1.0
    # partition-major pack: [BH, 128, 64, 65] (n = t*128 + pp)
    v65 = np.ascontiguousarray(
        v65.reshape(BH, N // 128, 128, 65).transpose(0, 2, 1, 3))
    gs = np.array([[1.0 / gmax]], np.float32)

    nc = _build()
    in_maps = []
    for c in range(NCORES):
        sl = slice(c * PAIRS, (c + 1) * PAIRS)
        lh = np.ascontiguousarray(
            landh[sl].transpose(1, 0, 2).reshape(128, PAIRS * 256))
        l32 = np.ascontiguousarray(
            land32[sl].transpose(1, 0, 2).reshape(64, PAIRS * 256))
        in_maps.append({"KQT": kqt[sl], "V65": v65[sl],
                        "LANDH": lh, "LAND32": l32, "GS": gs})
    res = run_bass_kernel_spmd(nc, in_maps, list(range(NCORES)))
    global LAST_RESULTS
    LAST_RESULTS = res
    xo = np.concatenate([res.results[c]["XO"] for c in range(NCORES)], axis=0)
    # unpack partition-major: [BH, 128, 64, 64] -> [BH, N, 64]
    xo = xo.transpose(0, 2, 1, 3).reshape(BH, N, D)
    return np.ascontiguousarray(xo).reshape(B, H, N, D).astype(np.float32)


# revision 21
# speedup vs baseline: 1.3287x; 1.0048x over previous
import numpy as np
import ml_dtypes

B, H, N, D, M = 4, 12, 8192, 64, 128
NCORES = 8
PAIRS = (B * H) // NCORES   # 6 pairs per core
NG2 = 8                     # 8 double-groups of 1024 per pair
SKEW = 3                    # phase-3 runs SKEW slots behind phase-1

_cache = {}


def _build():
    if "nc" in _cache:
        return _cache["nc"]
    import concourse.bacc as bacc
    import concourse.mybir as mybir
    import concourse.tile as tile

    f32 = mybir.dt.float32
    f16 = mybir.dt.float16
    bf16 = mybir.dt.bfloat16
    AF = mybir.ActivationFunctionType

    nc = bacc.Bacc("TRN2", target_bir_lowering=False, debug=False)
    # K^T on partitions 0:64, Q^T on 64:128
    KQT = nc.declare_dram_parameter("KQT", [PAIRS, 128, N], f16, isOutput=False)
    V65 = nc.declare_dram_parameter("V65", [PAIRS, 128, N // 128, 65], bf16, isOutput=False)
    # landmark tiles: cols 0:128 = nr^T, 128:256 = nc^T; rows 64:128 dup rows 0:64
    LANDH = nc.declare_dram_parameter("LANDH", [128, PAIRS * 256], f16, isOutput=False)
    LAND32 = nc.declare_dram_parameter("LAND32", [64, PAIRS * 256], f32, isOutput=False)
    GS = nc.declare_dram_parameter("GS", [1, 1], f32, isOutput=False)
    XO = nc.declare_dram_parameter("XO", [PAIRS, 128, N // 128, 64], bf16, isOutput=True)

    with tile.TileContext(nc) as tc:
        with (tc.tile_pool(name="pc", bufs=1) as pc,
              tc.tile_pool(name="pkq", bufs=5) as pkq,
              tc.tile_pool(name="pv", bufs=3) as pv,
              tc.tile_pool(name="pw", bufs=3) as pw,
              tc.tile_pool(name="pxs", bufs=6) as pxs,
              tc.tile_pool(name="pns", bufs=2) as pns,
              tc.tile_pool(name="ps_rt", bufs=1, space="PSUM") as ps_rt_pool,
              tc.tile_pool(name="ps_S", bufs=1, space="PSUM") as ps_S_pool,
              tc.tile_pool(name="ps_cm", bufs=1, space="PSUM") as ps_cm_pool,
              tc.tile_pool(name="ps_x", bufs=1, space="PSUM") as ps_x_pool,
              tc.tile_pool(name="ps_ns", bufs=1, space="PSUM") as ps_ns_pool):

            # ---- landmarks + bulk loads (landmarks first: m-chains/NS
            # prep runs while the big DMAs stream) ----
            gs_sb = pc.tile([1, 1], f32, tag="gs_sb")
            nc.sync.dma_start(gs_sb[:], GS[:])
            landh_all = pc.tile([128, PAIRS * 256], f16, tag="landh_all")
            nc.sync.dma_start(landh_all[:], LANDH[:])
            land32_all = pc.tile([64, PAIRS * 256], f32, tag="land32_all")
            nc.sync.dma_start(land32_all[:], LAND32[:])
            kq_tiles = [None] * PAIRS
            v_tiles = [None] * PAIRS

            def load_pair(p):
                kq = pkq.tile([128, N], f16, tag="kq", name="kq")
                nc.gpsimd.dma_start(kq[:], KQT[p])
                vt = pv.tile([128, N // 128, 65], bf16, tag="v65", name="vt")
                nc.gpsimd.dma_start(vt[:], V65[p])
                kq_tiles[p] = kq
                v_tiles[p] = vt

            load_pair(0)

            # ---- constants ----
            ident = pc.tile([128, 128], bf16, tag="ident")
            nc.gpsimd.memset(ident[:], 0.0)
            nc.gpsimd.affine_select(out=ident[:], in_=ident[:],
                compare_op=mybir.AluOpType.not_equal, fill=1.0, base=0,
                pattern=[[-1, 128]], channel_multiplier=1)
            diags = {}
            for val, tg in ((7.0, "i7"), (15.0, "i15"), (13.0, "i13")):
                t = pc.tile([128, PAIRS * 128], bf16, tag=tg, name=tg)
                nc.gpsimd.memset(t[:], 0.0)
                for p in range(PAIRS):
                    nc.gpsimd.affine_select(
                        out=t[:, p * 128:(p + 1) * 128],
                        in_=t[:, p * 128:(p + 1) * 128],
                        compare_op=mybir.AluOpType.not_equal, fill=val, base=0,
                        pattern=[[-1, 128]], channel_multiplier=1)
                diags[tg] = t
            i7b, i15b, i13b = diags["i7"], diags["i15"], diags["i13"]

            ones_row = pc.tile([1, 128], f32, tag="ones_row")
            nc.vector.memset(ones_row[:], 1.0)
            ps_bc = ps_cm_pool.tile([128, 1], f32, tag="ps_cm", name="ps_bc")
            nc.tensor.matmul(ps_bc[:], ones_row[:], gs_sb[:], start=True, stop=True)
            gsb = pc.tile([128, 1], f32, tag="gsb")
            nc.scalar.copy(gsb[:], ps_bc[:])

            load_pair(1)

            # ---- m-chains: k2 for all pairs into batched tiles ----
            W = PAIRS * 128
            k2_all = pc.tile([128, W], bf16, tag="k2_all")
            for p in range(PAIRS):
                ps_m = ps_cm_pool.tile([128, 128], f32, tag="ps_cm", name="ps_m")
                nc.tensor.matmul(ps_m[:], land32_all[:, p * 256: p * 256 + 128], land32_all[:, p * 256 + 128: p * 256 + 256],
                                 start=True, stop=True)
                e_m = pns.tile([128, 128], f32, tag="e_m", name="e_m")
                msum = pns.tile([128, 1], f32, tag="msum", name="msum")
                nc.scalar.activation(e_m[:], ps_m[:], AF.Exp, accum_out=msum[:])
                mrec = pns.tile([128, 1], f32, tag="mrec", name="mrec")
                nc.vector.reciprocal(mrec[:], msum[:])
                nc.vector.tensor_scalar_mul(k2_all[:, p * 128:(p + 1) * 128],
                                            e_m[:], mrec[:])
            ps_t = ps_ns_pool.tile([128, W], bf16, tag="ps_ns", name="ps_t")
            for p in range(PAIRS):
                nc.tensor.transpose(ps_t[:, p * 128:(p + 1) * 128],
                                    k2_all[:, p * 128:(p + 1) * 128], ident[:])
            k2t_all = pc.tile([128, W], bf16, tag="k2t_all")
            nc.scalar.copy(k2t_all[:], ps_t[:])
            vm_all = pns.tile([128, W], bf16, tag="vm", name="vm0")
            nc.vector.tensor_scalar_mul(vm_all[:], ps_t[:], gsb[:])
            vt_all = pns.tile([128, W], bf16, tag="vt", name="vt0")
            nc.vector.tensor_scalar_mul(vt_all[:], k2_all[:], gsb[:])

            ns_state = {"vm": vm_all, "vt": vt_all}

            # ---- batched NS stage thunks ----
            def ns_thunks():
                for _ in range(6):
                    carry = {}

                    def cA():
                        ps_P = ps_ns_pool.tile([128, W], f32, tag="ps_ns", name="ps_P")
                        vm = ns_state["vm"]
                        for p in range(PAIRS):
                            nc.tensor.matmul(ps_P[:, p * 128:(p + 1) * 128],
                                             k2t_all[:, p * 128:(p + 1) * 128],
                                             vm[:, p * 128:(p + 1) * 128],
                                             start=True, stop=True)
                        pbf = pns.tile([128, W], bf16, tag="pbf", name="pbf")
                        nc.scalar.copy(pbf[:], ps_P[:])
                        t1 = pns.tile([128, W], bf16, tag="t1", name="t1")
                        nc.vector.tensor_sub(t1[:], i7b[:], ps_P[:])
                        carry["pbf"] = pbf
                        carry["t1"] = t1

                    def cB():
                        ps_pt = ps_ns_pool.tile([128, W], bf16, tag="ps_ns", name="ps_pt")
                        for p in range(PAIRS):
                            nc.tensor.transpose(ps_pt[:, p * 128:(p + 1) * 128],
                                                carry["pbf"][:, p * 128:(p + 1) * 128],
                                                ident[:])
                        ptb = pns.tile([128, W], bf16, tag="ptb", name="ptb")
                        nc.scalar.copy(ptb[:], ps_pt[:])
                        carry["ptb"] = ptb

                    def cC():
                        ps_u = ps_ns_pool.tile([128, W], f32, tag="ps_ns", name="ps_u")
                        for p in range(PAIRS):
                            nc.tensor.matmul(ps_u[:, p * 128:(p + 1) * 128],
                                             carry["ptb"][:, p * 128:(p + 1) * 128],
                                             carry["t1"][:, p * 128:(p + 1) * 128],
                                             start=True, stop=True)
                        t2 = pns.tile([128, W], bf16, tag="t2", name="t2")
                        nc.vector.tensor_sub(t2[:], i15b[:], ps_u[:])
                        carry["t2"] = t2

                    def cD():
                        ps_w = ps_ns_pool.tile([128, W], f32, tag="ps_ns", name="ps_w")
                        for p in range(PAIRS):
                            nc.tensor.matmul(ps_w[:, p * 128:(p + 1) * 128],
                                             carry["ptb"][:, p * 128:(p + 1) * 128],
                                             carry["t2"][:, p * 128:(p + 1) * 128],
                                             start=True, stop=True)
                        t3 = pns.tile([128, W], bf16, tag="t3", name="t3")
                        nc.vector.tensor_sub(t3[:], i13b[:], ps_w[:])
                        carry["t3"] = t3

                    def cE():
                        vt = ns_state["vt"]
                        ps_v = ps_ns_pool.tile([128, W], f32, tag="ps_ns", name="ps_v")
                        for p in range(PAIRS):
                            nc.tensor.matmul(ps_v[:, p * 128:(p + 1) * 128],
                                             vt[:, p * 128:(p + 1) * 128],
                                             carry["t3"][:, p * 128:(p + 1) * 128],
                                             start=True, stop=True)
                        vm_n = pns.tile([128, W], bf16, tag="vm", name="vm_n")
                        nc.vector.tensor_scalar(vm_n[:], ps_v[:], 0.25, scalar2=None,
                                                op0=mybir.AluOpType.mult)
                        ps_vt = ps_ns_pool.tile([128, W], f32, tag="ps_ns", name="ps_vt")
                        for p in range(PAIRS):
                            nc.tensor.matmul(ps_vt[:, p * 128:(p + 1) * 128],
                                             carry["t3"][:, p * 128:(p + 1) * 128],
                                             vt[:, p * 128:(p + 1) * 128],
                                             start=True, stop=True)
                        vt_n = pns.tile([128, W], bf16, tag="vt", name="vt_n")
                        nc.vector.tensor_scalar(vt_n[:], ps_vt[:], 0.25, scalar2=None,
                                                op0=mybir.AluOpType.mult)
                        ns_state["vm"] = vm_n
                        ns_state["vt"] = vt_n

                    yield cA
                    yield cB
                    yield cC
                    yield cD
                    yield cE

            ns_iter = ns_thunks()
            # kick off first chunks before slot 0 (PE is idle during DMA warmup)
            for _ in range(4):
                th = next(ns_iter, None)
                if th is not None:
                    th()

            # ---- main pipelined slots: p1(s) s<6, p3(s-SKEW) s>=SKEW ----
            A1 = [None] * PAIRS
            ps_S_handle = [None] * PAIRS
            s_bf_handle = [None] * PAIRS
            ert_prev = [None]
            ec_prev = [None]
            xstage = [None]

            for s in range(PAIRS + SKEW):
                if 2 <= s + 1 <= PAIRS - 1:
                    load_pair(s + 1)
                for gg in range(NG2 + 1):
                    # p1: r^T matmuls (pair s, group gg)
                    if s < PAIRS and gg < NG2:
                        kq = kq_tiles[s]
                        ps_rt = ps_rt_pool.tile([128, 1024], f32, tag="ps_rt",
                                                name="ps_rt")
                        for j in range(8):
                            nc.tensor.matmul(
                                ps_rt[:, j * 128:(j + 1) * 128],
                                kq[0:64, gg * 1024 + j * 128: gg * 1024 + (j + 1) * 128],
                                landh_all[0:64, s * 256: s * 256 + 128],
                                start=True, stop=True)
                        ert = pw.tile([128, 1024], bf16, tag="ert", name="ert")
                        nc.scalar.activation(ert[:], ps_rt[:], AF.Exp)
                        ert_new = ert
                    # p3: first half X matmuls + divide (pair s-SKEW, group gg-1)
                    if s >= SKEW and gg >= 1:
                        pp = s - SKEW
                        gm = gg - 1
                        if gm % 2 == 0:
                            xstage[0] = pxs.tile([128, 16, 64], bf16, tag="xstage",
                                                 name="xstage")
                        ps_x0 = ps_x_pool.tile([128, 4, 65], f32, tag="ps_x",
                                               name="ps_x0")
                        for j in range(4):
                            nc.tensor.matmul(ps_x0[:, j, :],
                                             ec_prev[0][:, j * 128:(j + 1) * 128],
                                             A1[pp][:], start=True, stop=True)
                        xr0 = pw.tile([128, 4], f32, tag="xr0", name="xr0")
                        nc.vector.reciprocal(xr0[:], ps_x0[:, :, 64])
                        nc.vector.tensor_tensor(
                            out=xstage[0][:, (gm % 2) * 8:(gm % 2) * 8 + 4, :],
                            in0=ps_x0[:, :, 0:64],
                            in1=xr0.rearrange("p (t o) -> p t o", o=1).to_broadcast([128, 4, 64]),
                            op=mybir.AluOpType.mult)
                    # p3: c matmuls + exp (pair s-SKEW, group gg)
                    if s >= SKEW and gg < NG2:
                        pp = s - SKEW
                        kqp = kq_tiles[pp]
                        ps_c = ps_cm_pool.tile([128, 1024], f32, tag="ps_cm",
                                               name="ps_c")
                        for j in range(8):
                            nc.tensor.matmul(
                                ps_c[:, j * 128:(j + 1) * 128],
                                landh_all[64:128, pp * 256 + 128: pp * 256 + 256],
                                kqp[64:128, gg * 1024 + j * 128: gg * 1024 + (j + 1) * 128],
                                start=True, stop=True)
                        ec = pw.tile([128, 1024], bf16, tag="ec", name="ec")
                        nc.scalar.activation(ec[:], ps_c[:], AF.Exp)
                        ec_new = ec
                    # p1: S accumulation (pair s, group gg-1)
                    if s < PAIRS and gg >= 1:
                        if gg == 1:
                            ps_S = ps_S_pool.tile([128, 65], f32, tag="ps_S",
                                                  name="ps_S")
                            ps_S_handle[s] = ps_S
                        ps_S = ps_S_handle[s]
                        for j in range(8):
                            nc.tensor.matmul(
                                ps_S[:],
                                ert_prev[0][:, j * 128:(j + 1) * 128],
                                v_tiles[s][:, (gg - 1) * 8 + j, :],
                                start=(gg == 1 and j == 0),
                                stop=(gg == NG2 and j == 7),
                                skip_group_check=True)
                    # p3: second half X matmuls + divide + store
                    if s >= SKEW and gg >= 1:
                        pp = s - SKEW
                        gm = gg - 1
                        ps_x1 = ps_x_pool.tile([128, 4, 65], f32, tag="ps_x",
                                               name="ps_x1")
                        for j in range(4):
                            nc.tensor.matmul(ps_x1[:, j, :],
                                             ec_prev[0][:, 512 + j * 128: 512 + (j + 1) * 128],
                                             A1[pp][:], start=True, stop=True)
                        xr1 = pw.tile([128, 4], f32, tag="xr1", name="xr1")
                        nc.vector.reciprocal(xr1[:], ps_x1[:, :, 64])
                        nc.vector.tensor_tensor(
                            out=xstage[0][:, (gm % 2) * 8 + 4:(gm % 2) * 8 + 8, :],
                            in0=ps_x1[:, :, 0:64],
                            in1=xr1.rearrange("p (t o) -> p t o", o=1).to_broadcast([128, 4, 64]),
                            op=mybir.AluOpType.mult)
                        if gm % 2 == 1:
                            q = gm // 2
                            nc.sync.dma_start(
                                XO[pp, :, q * 16:(q + 1) * 16, :], xstage[0][:])
                    if s < PAIRS and gg < NG2:
                        ert_prev[0] = ert_new
                    if s >= SKEW and gg < NG2:
                        ec_prev[0] = ec_new
                    # drip-feed one NS stage per gg
                    th = next(ns_iter, None)
                    if th is not None:
                        th()

                # S normalization for pair s (frees ps_S promptly; no NS dep)
                if s < PAIRS:
                    ps_S = ps_S_handle[s]
                    rrec = pw.tile([128, 1], f32, tag="rrec", name="rrec")
                    nc.vector.reciprocal(rrec[:], ps_S[:, 64:65])
                    s_bf = pw.tile([128, 64], bf16, tag="s_bf", name="s_bf")
                    nc.vector.tensor_scalar_mul(s_bf[:], ps_S[:, 0:64], rrec[:])
                    s_bf_handle[s] = s_bf
                # A-matmul for pair s-(SKEW-1): needs final NS state
                if SKEW - 1 <= s < PAIRS + SKEW - 1:
                    pa = s - (SKEW - 1)
                    vt_fin = ns_state["vt"]
                    ps_A = ps_x_pool.tile([128, 64], f32, tag="ps_x", name="ps_A")
                    nc.tensor.matmul(ps_A[:], vt_fin[:, pa * 128:(pa + 1) * 128],
                                     s_bf_handle[pa][:], start=True, stop=True)
                    a1 = pw.tile([128, 65], bf16, tag="A1", bufs=3, name="a1")
                    nc.vector.memset(a1[:, 64:65], 1.0)
                    nc.vector.tensor_copy(a1[:, 0:64], ps_A[:])
                    A1[pa] = a1

    nc.finalize()
    _cache["nc"] = nc
    return nc


def kernel(Q, K, V, mask):
    from concourse.bass_utils import run_bass_kernel_spmd

    Q = np.asarray(Q, dtype=np.float32)
    K = np.asarray(K, dtype=np.float32)
    V = np.asarray(V, dtype=np.float32)
    BH = B * H
    Qf = Q.reshape(BH, N, D)
    Kf = K.reshape(BH, N, D)
    Vf = V.reshape(BH, N, D)

    # host: top-k selection + global NS init scalar
    landh = np.empty((BH, 128, 256), np.float16)
    land32 = np.empty((BH, 64, 256), np.float32)
    gmax = 0.0
    for i in range(BH):
        sK = Kf[i, :, 0].copy(); sK[0] = np.inf
        iK = np.sort(np.argpartition(-sK, M)[:M])
        sQ = Qf[i, :, 0].copy(); sQ[0] = np.inf
        iQ = np.sort(np.argpartition(-sQ, M)[:M])
        nr = Qf[i, iQ]          # [M, D]
        ncm = Kf[i, iK]         # [M, D]
        land32[i, :, 0:128] = nr.T
        land32[i, :, 128:256] = ncm.T
        landh[i, 0:64, 0:128] = nr.T
        landh[i, 0:64, 128:256] = ncm.T
        landh[i, 64:128, :] = landh[i, 0:64, :]
        md = nr.astype(np.float64) @ ncm.astype(np.float64).T
        e = np.exp(md - md.max(axis=1, keepdims=True))
        k2 = e / e.sum(axis=1, keepdims=True)
        gmax = max(gmax, float(k2.sum(axis=0).max()))

    kqt = np.empty((BH, 128, N), np.float16)
    kqt[:, 0:64, :] = Kf.transpose(0, 2, 1)
    kqt[:, 64:128, :] = Qf.transpose(0, 2, 1)
    v65 = np.empty((BH, N, 65), ml_dtypes.bfloat16)
    v65[:, :, 0:64] = Vf.astype(ml_dtypes.bfloat16)
    v65[:, :, 64] = 1.0
    # partition-major pack: [BH, 128, 64, 65] (n = t*128 + pp)
    v65 = np.ascontiguousarray(
        v65.reshape(BH, N // 128, 128, 65).transpose(0, 2, 1, 3))
    gs = np.array([[1.0 / gmax]], np.float32)

    nc = _build()
    in_maps = []
    for c in range(NCORES):
        sl = slice(c * PAIRS, (c + 1) * PAIRS)
        lh = np.ascontiguousarray(
            landh[sl].transpose(1, 0, 2).reshape(128, PAIRS * 256))
        l32 = np.ascontiguousarray(
            land32[sl].transpose(1, 0, 2).reshape(64, PAIRS * 256))
        in_maps.append({"KQT": kqt[sl], "V65": v65[sl],
                        "LANDH": lh, "LAND32": l32, "GS": gs})
    res = run_bass_kernel_spmd(nc, in_maps, list(range(NCORES)))
    global LAST_RESULTS
    LAST_RESULTS = res
    xo = np.concatenate([res.results[c]["XO"] for c in range(NCORES)], axis=0)
    # unpack partition-major: [BH, 128, 64, 64] -> [BH, N, 64]
    xo = xo.transpose(0, 2, 1, 3).reshape(BH, N, D)
    return np.ascontiguousarray(xo).reshape(B, H, N, D).astype(np.float32)
